# revision 56
# baseline (speedup 1.0000x reference)
"""Causal self-attention (k/q swapped variant) on 8 Trainium2 NeuronCores.

Problem (hardcoded shapes): B=2, N=2048, D=1024, H=16, DH=64.
  kqv = einsum('bnd,hde->bhne', x, Wkqv) + bkqv   ; split -> k, q, v
  A[b,h,n,m] = k[b,h,n]·q[b,h,m] / sqrt(DH), causal mask m<=n, softmax over m
  sa = A @ v ; concat heads ; out = sa @ Wo + bo

Sharding: batch x heads — core c owns batch c//4 and heads 4*(c%4)..+4 (two
head-pairs A/B), computes its partial output projection sa_local @ Wo[rows]
in bf16 over its single batch, and the host sums 4 partials per batch (+bo)
in fp32. This halves both the x input DMA and the partial-output DMA vs
all-batches-per-core head sharding.

Per-core device kernel (all matmul operands bf16, fp32 PSUM accumulation):
  - x is pre-transposed on host to xt = x[b].T ([D, N]) so the contraction
    dim d lands on SBUF partitions; one copy shared by both head-pairs.
  - k/q are projected with W-stationary matmuls into [dh, n] layout; v is
    projected directly into [n, dh] layout with xt-stationary matmuls (one
    matmul covers all 4 heads), so no PE transposes / extra copies at all.
  - scores are computed transposed, S^T[m, n] = q[m]·k[n], so softmax's
    reduction dim m sits on partitions; both heads of a pair live in ONE
    [128, 1024] PSUM tile (2 banks) so off-diagonal chunks need a single wide
    exp() on the ACT engine (diag chunks use one strided-AP exp). The
    denominator comes free from the PV matmul: v blocks are [v_h|ones] so
    PV psum rows 64:128 hold the softmax denominators for both heads.
  - the chunk loop is software-pipelined: scores+exp of chunk ci+1 are
    emitted before the PV of chunk ci; k/q projection and output-projection
    slots are woven between chunks; v-block projections are emitted inline
    in pair-0 blocks right where the PE would otherwise wait for exp.
  - pair-1 attention blocks run in DESCENDING j order so the final block is
    the 4-chunk j=0 and the tail (last norm + out-blocks 0..3) is short.
  - PSUM budget (8 banks): scores 2x[128,1024] (4) + PV accumulator
    [128,1024] (2) + proj/outproj bank 1 + v-proj/outproj bank 1.
"""

import numpy as np
import ml_dtypes

B = 2
N = 2048
D = 1024
H = 16
DH = 64
NCORES = 8
HL = 2                    # heads per pair
NPAIR = 2                 # head-pairs per core
DC = D // 128             # contraction chunks = 8
NB = N // 128             # 128-row blocks = 16
NJ = N // 512             # 512-col blocks = 4

BF16 = ml_dtypes.bfloat16

_CACHE = {}


def _build():
    import concourse.bass as bass
    import concourse.mybir as mybir
    import concourse.tile as tile
    from concourse import bacc
    from contextlib import ExitStack

    f32 = mybir.dt.float32
    bf16 = mybir.dt.bfloat16
    Exp = mybir.ActivationFunctionType.Exp

    nc = bacc.Bacc("TRN2", target_bir_lowering=False, debug=False,
                   enable_asserts=False, num_devices=NCORES)

    xt_d = nc.dram_tensor("xt", [D, N], bf16, kind="ExternalInput")
    # k/q weights arrive pre-shuffled to the SBUF layout [128, DC*128]
    # (partition = within-chunk row, free = (chunk, head-col)), per pair
    wk_d = {p: nc.dram_tensor(f"wk{p}", [128, DC * 128], bf16,
                              kind="ExternalInput") for p in range(NPAIR)}
    wq_d = {p: nc.dram_tensor(f"wq{p}", [128, DC * 128], bf16,
                              kind="ExternalInput") for p in range(NPAIR)}
    # v weights combined for both pairs: per chunk dc the 256 cols are
    # [pair0 h0|h1 | pair1 h0|h1]
    wvall_d = nc.dram_tensor("wvall", [128, DC * 256], bf16,
                             kind="ExternalInput")
    wo_d = {p: nc.dram_tensor(f"wo{p}", [128, D], bf16,
                              kind="ExternalInput") for p in range(NPAIR)}
    # packed f32 consts: [bvr0(128) | bvr1(128) | bk0 | bq0 | bk1 | bq1]
    cf32_d = nc.dram_tensor("cf32", [128, 260], f32, kind="ExternalInput")
    m01_d = nc.dram_tensor("m01", [128, 128], bf16, kind="ExternalInput")
    out_d = nc.dram_tensor("out", [N, D], bf16, kind="ExternalOutput")

    with tile.TileContext(nc) as tc, ExitStack() as ctx:
        const = ctx.enter_context(tc.tile_pool(name="const", bufs=1))
        xt_pool = ctx.enter_context(tc.tile_pool(name="xt", bufs=1))
        kq_pool = ctx.enter_context(tc.tile_pool(name="kq", bufs=4))
        v_pool = ctx.enter_context(tc.tile_pool(name="v", bufs=2))
        sa_pool = ctx.enter_context(tc.tile_pool(name="sa", bufs=2))
        pt_pool = ctx.enter_context(tc.tile_pool(name="pt", bufs=6))
        rc_pool = ctx.enter_context(tc.tile_pool(name="rc", bufs=2))
        ob_pool = ctx.enter_context(tc.tile_pool(name="ob", bufs=4))
        s_ps = ctx.enter_context(tc.tile_pool(name="s_ps", bufs=2, space="PSUM"))
        pv_ps = ctx.enter_context(tc.tile_pool(name="pv_ps", bufs=1, space="PSUM"))
        wv_ps = ctx.enter_context(tc.tile_pool(name="wv_ps", bufs=1, space="PSUM"))
        tp_ps = ctx.enter_context(tc.tile_pool(name="tp_ps", bufs=1, space="PSUM"))

        wk_sb, wq_sb, wo_sb = {}, {}, {}
        for p in range(NPAIR):
            wk_sb[p] = const.tile([128, DC * 128], bf16, name=f"wk_sb{p}")
            wq_sb[p] = const.tile([128, DC * 128], bf16, name=f"wq_sb{p}")
            wo_sb[p] = const.tile([128, D], bf16, name=f"wo_sb{p}")
        wvall_sb = const.tile([128, DC * 256], bf16, name="wvall_sb")
        cf32_sb = const.tile([128, 260], f32, name="cf32_sb")
        m01_sb = const.tile([128, 128], bf16, name="m01_sb")
        bvr = {p: cf32_sb[:, 128 * p:128 * p + 128] for p in range(NPAIR)}
        bk = {p: cf32_sb[:, 256 + 2 * p:257 + 2 * p] for p in range(NPAIR)}
        bq = {p: cf32_sb[:, 257 + 2 * p:258 + 2 * p] for p in range(NPAIR)}

        # ---- DMA issue order: what the start of the pipeline needs first.
        # kqv0 needs wk0 + xt q0; the first dvv needs wvall + bvr; the first
        # diag masks need m01; q1 feeds the first weave slots. Each xt
        # quadrant is ONE multi-dim DMA ([p, dc, c] pattern) — per-DMA ring
        # time (~0.6us) dominates piece transfers, so fewer, bigger is
        # faster. q0 is split in two (dc 0-3 / 4-7 over both rings) so the
        # kqv0 chain can start sooner.
        xtq = {}   # quad -> [128, DC*512] tile (chunk dc at cols dc*512)

        def xt_quad(q, eng, dc0=0, dc1=DC, tile_=None):
            t = tile_
            if t is None:
                t = xt_pool.tile([128, DC * 512], bf16, name=f"xt_q{q}",
                                 tag="xt", bufs=4)
                xtq[q] = t
            eng.dma_start(
                t[:].rearrange("p (dc c) -> p dc c", c=512)[:, dc0:dc1, :],
                xt_d.ap()[:, q * 512:(q + 1) * 512]
                .rearrange("(dc p) c -> p dc c", p=128)[:, dc0:dc1, :])
            return t

        nc.sync.dma_start(wk_sb[0][:], wk_d[0].ap())
        t0 = xt_quad(0, nc.sync, 0, 4)
        xt_quad(0, nc.scalar, 4, DC, tile_=t0)
        nc.scalar.dma_start(wq_sb[0][:], wq_d[0].ap())
        nc.scalar.dma_start(cf32_sb[:], cf32_d.ap())
        nc.sync.dma_start(m01_sb[:], m01_d.ap())
        t1 = xt_quad(1, nc.sync, 0, 4)
        xt_quad(1, nc.scalar, 4, DC, tile_=t1)
        nc.scalar.dma_start(wvall_sb[:], wvall_d.ap())
        t2 = xt_quad(2, nc.sync, 0, 4)
        xt_quad(2, nc.scalar, 4, DC, tile_=t2)
        nc.sync.dma_start(wk_sb[1][:], wk_d[1].ap())
        nc.scalar.dma_start(wq_sb[1][:], wq_d[1].ap())
        t3 = xt_quad(3, nc.sync, 0, 4)
        xt_quad(3, nc.scalar, 4, DC, tile_=t3)
        nc.sync.dma_start(wo_sb[0][:], wo_d[0].ap())
        nc.scalar.dma_start(wo_sb[1][:], wo_d[1].ap())

        def xt_ap(dc, c0, c1):
            q = c0 // 512
            return xtq[q][:, dc * 512 + c0 - q * 512:dc * 512 + c1 - q * 512]

        # ---- per-pair tensors
        k2 = {p: kq_pool.tile([128, N], bf16, name=f"k2_p{p}", tag="kq")
              for p in range(NPAIR)}
        q2 = {p: kq_pool.tile([128, N], bf16, name=f"q2_p{p}", tag="kq")
              for p in range(NPAIR)}
        # v blocks per nb: [ones(64) | v_h0(64) | ones(64) | v_h1(64)] so the
        # PV lhsT for head h is the contiguous 128 cols at nb*256 + 128h and
        # psum rows 0:64 give the denominator for BOTH heads (base-0 rows:
        # reciprocal_approx_fast is a custom-DVE op and misreads partition-
        # offset operands on HW).
        v_sb = {}
        sa_sb = {}
        groups = {p: ((wk_sb[p], bk[p], k2[p]), (wq_sb[p], bq[p], q2[p]))
                  for p in range(NPAIR)}

        for p in range(NPAIR):
            v_sb[p] = v_pool.tile([128, NB * 256], bf16, name=f"v_p{p}",
                                  tag="v")
            sa_sb[p] = sa_pool.tile([128, N], bf16, name=f"sa_p{p}", tag="sa")
            nc.gpsimd.memset(
                v_sb[p][:].rearrange("p (nb h q) -> p nb h q", h=2, q=128)
                [:, :, :, 0:64], 1.0)

        def make_proj_slots(p, gi, nj):
            """One [128, 512] k/q projection group as two weave slots (dc 0-3
            and dc 4-7 + bias add), sharing the wv PSUM bank."""
            st = {}

            def part(d0, d1):
                def go():
                    if d0 == 0:
                        st["ps"] = wv_ps.tile([128, 512], f32, name="wvps",
                                              tag="wv")
                    ps = st["ps"]
                    w_sb, bias_ap, dst = groups[p][gi]
                    for dc in range(d0, d1):
                        nc.tensor.matmul(
                            ps[:], w_sb[:, dc * 128:(dc + 1) * 128],
                            xt_ap(dc, nj * 512, (nj + 1) * 512),
                            start=(dc == 0), stop=(dc == DC - 1))
                    if d1 == DC:
                        nc.vector.tensor_scalar_add(
                            dst[:, nj * 512:(nj + 1) * 512], ps[:], bias_ap)
                return go
            return [part(0, 4), part(4, DC)]

        def make_dvv(nb):
            """v projection for one 128-row block, all 4 heads in one
            xt-stationary matmul chain; strided DVE write adds the bias and
            drops the result into the [v|ones|v|ones] layout."""
            vps = tp_ps.tile([128, 256], f32, name="vps", tag="tp",
                             padded_shape=[128, 512])
            q, r = nb // 4, nb % 4
            for dc in range(DC):
                nc.tensor.matmul(vps[:],
                                 xtq[q][:, dc * 512 + r * 128:
                                         dc * 512 + (r + 1) * 128],
                                 wvall_sb[:, dc * 256:(dc + 1) * 256],
                                 start=(dc == 0), stop=(dc == DC - 1))
            for p in range(NPAIR):
                nc.vector.tensor_tensor(
                    v_sb[p][:, nb * 256:(nb + 1) * 256]
                    .rearrange("p (h q2) -> p h q2", q2=128)[:, :, 64:128],
                    vps[:, 128 * p:128 * p + 128]
                    .rearrange("p (h e) -> p h e", e=64),
                    bvr[p].rearrange("p (h e) -> p h e", e=64),
                    mybir.AluOpType.add)

        def make_op_slot(nb, dma_eng=None, cast_eng=None):
            """Output projection of one 128-row block: both pairs accumulate;
            the two column halves alternate over the wv / tp banks so casts
            overlap the next matmuls; one [128, 1024] bf16 DMA per block.
            Casts default to DVE; tail blocks use ACT (idle by then)."""
            def go():
                ob = ob_pool.tile([128, 1024], bf16, name="ob", tag="ob")
                for half in range(2):
                    pool, tag = (wv_ps, "wv") if half == 0 else (tp_ps, "tp")
                    op = pool.tile([128, 512], f32, name="opps", tag=tag,
                                   padded_shape=[128, 512])
                    for p in range(NPAIR):
                        nc.tensor.matmul(
                            op[:], sa_sb[p][:, nb * 128:(nb + 1) * 128],
                            wo_sb[p][:, half * 512:(half + 1) * 512],
                            start=(p == 0), stop=(p == NPAIR - 1))
                    ce = cast_eng
                    if ce == "mix":
                        ce = nc.vector if half == 0 else nc.scalar
                    if ce is nc.scalar:
                        nc.scalar.copy(ob[:, half * 512:(half + 1) * 512],
                                       op[:])
                    else:
                        nc.vector.tensor_copy(
                            ob[:, half * 512:(half + 1) * 512], op[:])
                (dma_eng or nc.sync).dma_start(
                    out_d.ap()[nb * 128:(nb + 1) * 128, :], ob[:])
            return go

        def kqv0_start():
            """Pair-A k/q projections for nj=0, d-chunk-major so the PE is
            paced by the q0 xt piece DMAs; all k matmuls first so the PE can
            start before wq0 lands (k/q in one score-pool tile)."""
            kq0s = s_ps.tile([128, 1024], f32, name="kq0s", tag="s")
            for dc in range(DC):
                nc.tensor.matmul(kq0s[:, 0:512],
                                 wk_sb[0][:, dc * 128:(dc + 1) * 128],
                                 xt_ap(dc, 0, 512),
                                 start=(dc == 0), stop=(dc == DC - 1))
            nc.vector.tensor_scalar_add(k2[0][:, 0:512], kq0s[:, 0:512],
                                        bk[0])
            for dc in range(DC):
                nc.tensor.matmul(kq0s[:, 512:1024],
                                 wq_sb[0][:, dc * 128:(dc + 1) * 128],
                                 xt_ap(dc, 0, 512),
                                 start=(dc == 0), stop=(dc == DC - 1))
            nc.vector.tensor_scalar_add(q2[0][:, 0:512], kq0s[:, 512:1024],
                                        bq[0])

        def emit_scores(p, j, ci, state, mask_eng=None):
            """Scores (both heads into one 2-bank psum tile) + exp for one
            128-m chunk."""
            t = ci - 4 * j
            lo = 128 * t if t >= 0 else 0
            sp = s_ps.tile([128, 1024], f32, name="s", tag="s")
            pt = pt_pool.tile([128, 1024], bf16, name="pt", tag="pt")
            for h in range(HL):
                hp = 64 * h
                nc.tensor.matmul(
                    sp[:, 512 * h + lo:512 * h + 512],
                    q2[p][hp:hp + 64, ci * 128:(ci + 1) * 128],
                    k2[p][hp:hp + 64, j * 512 + lo:(j + 1) * 512],
                    start=True, stop=True)
            if t < 0:
                nc.scalar.activation(pt[:], sp[:], Exp, scale=0.125)
            else:
                # one strided activation covering both heads' valid region
                nc.scalar.activation(
                    pt[:].rearrange("p (h w) -> p h w", h=HL)[:, :, lo:512],
                    sp[:].rearrange("p (h w) -> p h w", h=HL)[:, :, lo:512],
                    Exp, scale=0.125)
                for h in range(HL):
                    # bf16 in/out: cheap on DVE; the last block's masks go to
                    # Pool instead so they don't queue behind the previous
                    # block's norm chain on DVE (bf16-only Pool ops are fine)
                    (mask_eng or nc.vector).tensor_tensor(
                        pt[:, 512 * h + lo:512 * h + lo + 128],
                        pt[:, 512 * h + lo:512 * h + lo + 128],
                        m01_sb[:], mybir.AluOpType.mult)
            state[ci] = (pt, lo)

        def emit_pv(p, j, ci, pv2, state, nch):
            pt, lo = state.pop(ci)
            for h in range(HL):
                nc.tensor.matmul(
                    pv2[:, 512 * h + lo:512 * (h + 1)],
                    v_sb[p][:, ci * 256 + 128 * h:ci * 256 + 128 * h + 128],
                    pt[:, 512 * h + lo:512 * h + 512],
                    start=(ci == 0), stop=(ci == nch - 1))

        def att_norm(p, j, pv2, last=False):
            # psum rows 0:64 = denominator, 64:128 = sa, for BOTH heads.
            # ONE full-tile copy (DVE cost is free-size only, so copying all
            # 128 rows costs the same as 64) frees the PV psum in ~1.25us so
            # the next block's PV isn't WAR-blocked; the mults become
            # SBUF-only and run on the Pool engine, keeping the DVE queue
            # short ahead of the next block's diag masks.
            # denominators are sums of exp() in [~2e-3, ~3e3]: inside
            # approx_fast's domain; 18-bit accuracy is far below the bf16
            # noise of the P*V numerator. (approx_fast misreads PSUM
            # operands on HW - bounce through SBUF first.)
            den_sb = rc_pool.tile([64, 1024], f32, name="den", tag="den")
            nc.vector.tensor_copy(den_sb[:], pv2[0:64, :])
            if last:
                # no successor uses the pv psum bank: read sa straight from
                # PSUM (exempt from the same-start-partition rule), saving
                # one copy on the tail critical path
                saf = pv2[64:128, :]
            else:
                saf_t = rc_pool.tile([64, 1024], f32, name="saf", tag="saf")
                nc.vector.tensor_copy(saf_t[:], pv2[64:128, :])
                saf = saf_t[:]
            rc = rc_pool.tile([64, 1024], f32, name="rc", tag="rc")
            nc.vector.reciprocal_approx_fast(rc[:], den_sb[:])
            for h in range(HL):
                # fp32-in/bf16-out tensor_tensor gives wrong results on the
                # Pool engine on HW (sim is fine) - keep these on DVE; all
                # SBUF inputs are base-partition-0 (verifier requirement,
                # and custom-DVE ops misread partition-offset operands)
                nc.vector.tensor_tensor(
                    sa_sb[p][64 * h:64 * h + 64, j * 512:(j + 1) * 512],
                    saf[:, 512 * h:512 * (h + 1)],
                    rc[:, 512 * h:512 * (h + 1)], mybir.AluOpType.mult)

        def att_j(p, j, weave=(), dv=False, wd=0, late=None, defer_norm=False,
                  mask_eng=None, last=False):
            """One attention j-block, software-pipelined: scores/exp of chunk
            ci+1 are emitted before PV of chunk ci; weave slots fill PE gaps
            (starting at chunk `wd` — op-slot weaves depend on the previous
            block's att_norm, so give that DVE chain a head start); in pair-0
            blocks the v projections for this j's rows are emitted inline
            where the PE would wait for exp. `late` (e.g. the previous
            block's deferred norm) is emitted after chunk 1 so this block's
            first diag masks aren't queued behind it on DVE. With
            `defer_norm` the caller gets a closure to emit this block's norm
            later."""
            pv2 = pv_ps.tile([128, 1024], f32, name="pv2", tag="pv")
            weave = list(weave)
            nch = 4 * (j + 1)
            state = {}
            emitted = 0
            emit_scores(p, j, 0, state, mask_eng=mask_eng)
            for ci in range(nch):
                if ci + 1 < nch:
                    emit_scores(p, j, ci + 1, state, mask_eng=mask_eng)
                if ci == 1 and late is not None:
                    late()
                if ci >= wd:
                    target = len(weave) * (ci + 1 - wd) // (nch - wd)
                    while emitted < target:
                        weave[emitted]()
                        emitted += 1
                if dv and ci < 4:
                    make_dvv(4 * j + ci)
                emit_pv(p, j, ci, pv2, state, nch)
            # norm first so its DVE chain heads the queue; leftover weave
            # slots (PE/ACT work) then overlap it
            if defer_norm:
                norm = [lambda: att_norm(p, j, pv2, last=last)]
            else:
                att_norm(p, j, pv2, last=last)
                norm = []
            while emitted < len(weave):
                weave[emitted]()
                emitted += 1
            return norm[0] if norm else None

        # ================= emission schedule =================
        def ksl(p, nj):
            return make_proj_slots(p, 0, nj)

        def qsl(p, nj):
            return make_proj_slots(p, 1, nj)

        def ops(nbs, cast_eng=None):
            return [make_op_slot(nb, cast_eng=cast_eng) for nb in nbs]

        kqv0_start()
        att_j(0, 0, weave=ksl(0, 1) + qsl(0, 1), dv=True)
        att_j(0, 1, weave=ksl(0, 2) + qsl(0, 2), dv=True)
        # (1,3)'s scores read ALL of q2[1], so every qsl(1,*) must precede
        # it; only the k-blocks of later pair-1 js may ride inside it
        att_j(0, 2, weave=ksl(0, 3) + qsl(0, 3) + qsl(1, 0) + qsl(1, 1),
              dv=True)
        att_j(0, 3, weave=qsl(1, 2) + qsl(1, 3) + ksl(1, 3) + ksl(1, 2),
              dv=True)
        # pair-1 j-blocks DESCEND so the last block (and tail) is small;
        # ops() weaves start wd chunks in: the previous block's att_norm
        # (whose sa they read) is a ~5.6us DVE chain behind the last diag
        # masks, so firing an op matmul earlier stalls the in-order PE queue
        att_j(1, 3, weave=ksl(1, 1) + ksl(1, 0))
        att_j(1, 2, weave=ops(range(12, 16)), wd=5)
        att_j(1, 1, weave=ops(range(8, 12)), wd=6)
        # wd=4 > nch pushes the whole weave to the post-norm flush: op(4..7)
        # matmuls (ACT casts) overlap norm(1,0) on DVE
        att_j(1, 0, weave=ops(range(4, 8), cast_eng="mix"), wd=4,
              mask_eng=nc.gpsimd, last=True)
        # tail: alternate the last out-DMAs over the two HW-DGE queues and
        # split casts across DVE/ACT so neither serializes the drain
        for nb, eng in zip(range(0, 4),
                           (nc.sync, nc.scalar, nc.sync, nc.scalar)):
            make_op_slot(nb, dma_eng=eng, cast_eng="mix")()

    nc.compile()
    return nc


def _get_nc():
    if "nc" not in _CACHE:
        _CACHE["nc"] = _build()
    return _CACHE["nc"]


def _prep_inputs(x, Wkqv, bkqv, Wo, bo):
    """Host-side shard prep: one input map per core (core c: batch c//4,
    head-pairs (4*(c%4), 4*(c%4)+1) and (+2, +3))."""
    tri = np.triu(np.ones((128, 128), np.float32)).astype(BF16)  # m' <= n''
    xts = [np.ascontiguousarray(x[b].T).astype(BF16) for b in range(B)]

    def shuf(w):
        # [D, 128] -> [128, DC*128]: partition = within-chunk row
        return np.ascontiguousarray(
            w.reshape(DC, 128, 128).transpose(1, 0, 2).reshape(128, DC * 128))

    in_maps = []
    for c in range(NCORES):
        b, m = c // 4, c % 4
        im = {"xt": xts[b], "m01": tri}
        wvs = {}
        cf32 = np.zeros((128, 260), np.float32)
        for p in range(NPAIR):
            h0 = 4 * m + 2 * p
            h1 = h0 + 1
            im[f"wk{p}"] = shuf(np.concatenate(
                [Wkqv[h0, :, 0:64], Wkqv[h1, :, 0:64]], axis=1)).astype(BF16)
            im[f"wq{p}"] = shuf(np.concatenate(
                [Wkqv[h0, :, 64:128], Wkqv[h1, :, 64:128]], axis=1)).astype(BF16)
            wvs[p] = shuf(np.concatenate(
                [Wkqv[h0, :, 128:192], Wkqv[h1, :, 128:192]], axis=1)).astype(BF16)
            im[f"wo{p}"] = Wo[64 * h0:64 * h0 + 128, :].astype(BF16)
            bv_pair = np.concatenate([bkqv[h0, 128:192], bkqv[h1, 128:192]])
            cf32[:, 128 * p:128 * p + 128] = bv_pair[None, :]
            cf32[:, 256 + 2 * p] = np.concatenate(
                [bkqv[h0, 0:64], bkqv[h1, 0:64]])
            cf32[:, 257 + 2 * p] = np.concatenate(
                [bkqv[h0, 64:128], bkqv[h1, 64:128]])
        # combined v weights: per chunk dc, 256 cols = [pair0 128 | pair1 128]
        wvall = np.zeros((128, DC * 256), np.float32).astype(BF16)
        for dc in range(DC):
            wvall[:, dc * 256:dc * 256 + 128] = wvs[0][:, dc * 128:(dc + 1) * 128]
            wvall[:, dc * 256 + 128:dc * 256 + 256] = wvs[1][:, dc * 128:(dc + 1) * 128]
        im["wvall"] = wvall
        im["cf32"] = cf32
        in_maps.append(im)
    return in_maps


def kernel(x, Wkqv, bkqv, Wo, bo):
    from concourse import bass_utils

    nc = _get_nc()
    in_maps = _prep_inputs(np.asarray(x), np.asarray(Wkqv), np.asarray(bkqv),
                           np.asarray(Wo), np.asarray(bo))
    res = bass_utils.run_bass_kernel_spmd(nc, in_maps, core_ids=list(range(NCORES)))
    acc = np.zeros((B, N, D), np.float32)
    for c in range(NCORES):
        acc[c // 4] += np.asarray(res.results[c]["out"], dtype=np.float32)
    acc += np.asarray(bo)[None, None, :]
    return acc


# revision 59
# speedup vs baseline: 1.0286x; 1.0286x over previous
"""Causal self-attention (k/q swapped variant) on 8 Trainium2 NeuronCores.

Problem (hardcoded shapes): B=2, N=2048, D=1024, H=16, DH=64.
  kqv = einsum('bnd,hde->bhne', x, Wkqv) + bkqv   ; split -> k, q, v
  A[b,h,n,m] = k[b,h,n]·q[b,h,m] / sqrt(DH), causal mask m<=n, softmax over m
  sa = A @ v ; concat heads ; out = sa @ Wo + bo

Sharding: batch x heads — core c owns batch c//4 and heads 4*(c%4)..+4 (two
head-pairs A/B), computes its partial output projection sa_local @ Wo[rows]
in bf16 over its single batch, and the host sums 4 partials per batch (+bo)
in fp32. This halves both the x input DMA and the partial-output DMA vs
all-batches-per-core head sharding.

Per-core device kernel (all matmul operands bf16, fp32 PSUM accumulation):
  - x is pre-transposed on host to xt = x[b].T ([D, N]) so the contraction
    dim d lands on SBUF partitions; one copy shared by both head-pairs.
  - k/q are projected with W-stationary matmuls into [dh, n] layout; v is
    projected directly into [n, dh] layout with xt-stationary matmuls (one
    matmul covers all 4 heads), so no PE transposes / extra copies at all.
  - scores are computed transposed, S^T[m, n] = q[m]·k[n], so softmax's
    reduction dim m sits on partitions; both heads of a pair live in ONE
    [128, 1024] PSUM tile (2 banks) so off-diagonal chunks need a single wide
    exp() on the ACT engine (diag chunks use one strided-AP exp). The
    denominator comes free from the PV matmul: v blocks are [v_h|ones] so
    PV psum rows 64:128 hold the softmax denominators for both heads.
  - the chunk loop is software-pipelined: scores+exp of chunk ci+1 are
    emitted before the PV of chunk ci; k/q projection and output-projection
    slots are woven between chunks; v-block projections are emitted inline
    in pair-0 blocks right where the PE would otherwise wait for exp.
  - pair-1 attention blocks run in DESCENDING j order so the final block is
    the 4-chunk j=0 and the tail (last norm + out-blocks 0..3) is short.
  - PSUM budget (8 banks): scores 2x[128,1024] (4) + PV accumulator
    [128,1024] (2) + proj/outproj bank 1 + v-proj/outproj bank 1.
"""

import numpy as np
import ml_dtypes

B = 2
N = 2048
D = 1024
H = 16
DH = 64
NCORES = 8
HL = 2                    # heads per pair
NPAIR = 2                 # head-pairs per core
DC = D // 128             # contraction chunks = 8
NB = N // 128             # 128-row blocks = 16
NJ = N // 512             # 512-col blocks = 4

BF16 = ml_dtypes.bfloat16

_CACHE = {}


def _build():
    import concourse.bass as bass
    import concourse.mybir as mybir
    import concourse.tile as tile
    from concourse import bacc
    from contextlib import ExitStack

    f32 = mybir.dt.float32
    bf16 = mybir.dt.bfloat16
    Exp = mybir.ActivationFunctionType.Exp

    nc = bacc.Bacc("TRN2", target_bir_lowering=False, debug=False,
                   enable_asserts=False, num_devices=NCORES)

    xt_d = nc.dram_tensor("xt", [D, N], bf16, kind="ExternalInput")
    # k/q weights arrive pre-shuffled to the SBUF layout [128, DC*128]
    # (partition = within-chunk row, free = (chunk, head-col)), per pair
    wk_d = {p: nc.dram_tensor(f"wk{p}", [128, DC * 128], bf16,
                              kind="ExternalInput") for p in range(NPAIR)}
    wq_d = {p: nc.dram_tensor(f"wq{p}", [128, DC * 128], bf16,
                              kind="ExternalInput") for p in range(NPAIR)}
    # v weights combined for both pairs: per chunk dc the 256 cols are
    # [pair0 h0|h1 | pair1 h0|h1]
    wvall_d = nc.dram_tensor("wvall", [128, DC * 256], bf16,
                             kind="ExternalInput")
    wo_d = {p: nc.dram_tensor(f"wo{p}", [128, D], bf16,
                              kind="ExternalInput") for p in range(NPAIR)}
    # packed f32 consts: [bvr0(128) | bvr1(128) | bk0 | bq0 | bk1 | bq1]
    cf32_d = nc.dram_tensor("cf32", [128, 260], f32, kind="ExternalInput")
    m01_d = nc.dram_tensor("m01", [128, 128], bf16, kind="ExternalInput")
    out_d = nc.dram_tensor("out", [N, D], bf16, kind="ExternalOutput")

    with tile.TileContext(nc) as tc, ExitStack() as ctx:
        const = ctx.enter_context(tc.tile_pool(name="const", bufs=1))
        xt_pool = ctx.enter_context(tc.tile_pool(name="xt", bufs=1))
        kq_pool = ctx.enter_context(tc.tile_pool(name="kq", bufs=4))
        v_pool = ctx.enter_context(tc.tile_pool(name="v", bufs=2))
        sa_pool = ctx.enter_context(tc.tile_pool(name="sa", bufs=2))
        pt_pool = ctx.enter_context(tc.tile_pool(name="pt", bufs=6))
        rc_pool = ctx.enter_context(tc.tile_pool(name="rc", bufs=2))
        ob_pool = ctx.enter_context(tc.tile_pool(name="ob", bufs=4))
        s_ps = ctx.enter_context(tc.tile_pool(name="s_ps", bufs=2, space="PSUM"))
        pv_ps = ctx.enter_context(tc.tile_pool(name="pv_ps", bufs=1, space="PSUM"))
        wv_ps = ctx.enter_context(tc.tile_pool(name="wv_ps", bufs=1, space="PSUM"))
        tp_ps = ctx.enter_context(tc.tile_pool(name="tp_ps", bufs=1, space="PSUM"))

        wk_sb, wq_sb, wo_sb = {}, {}, {}
        for p in range(NPAIR):
            wk_sb[p] = const.tile([128, DC * 128], bf16, name=f"wk_sb{p}")
            wq_sb[p] = const.tile([128, DC * 128], bf16, name=f"wq_sb{p}")
            wo_sb[p] = const.tile([128, D], bf16, name=f"wo_sb{p}")
        wvall_sb = const.tile([128, DC * 256], bf16, name="wvall_sb")
        cf32_sb = const.tile([128, 260], f32, name="cf32_sb")
        m01_sb = const.tile([128, 128], bf16, name="m01_sb")
        bvr = {p: cf32_sb[:, 128 * p:128 * p + 128] for p in range(NPAIR)}
        bk = {p: cf32_sb[:, 256 + 2 * p:257 + 2 * p] for p in range(NPAIR)}
        bq = {p: cf32_sb[:, 257 + 2 * p:258 + 2 * p] for p in range(NPAIR)}

        # ---- DMA issue order: what the start of the pipeline needs first.
        # kqv0 needs wk0 + xt q0; the first dvv needs wvall + bvr; the first
        # diag masks need m01; q1 feeds the first weave slots. Each xt
        # quadrant is ONE multi-dim DMA ([p, dc, c] pattern) — per-DMA ring
        # time (~0.6us) dominates piece transfers, so fewer, bigger is
        # faster. q0 is split in two (dc 0-3 / 4-7 over both rings) so the
        # kqv0 chain can start sooner.
        xtq = {}   # quad -> [128, DC*512] tile (chunk dc at cols dc*512)

        def xt_quad(q, eng, dc0=0, dc1=DC, tile_=None):
            t = tile_
            if t is None:
                t = xt_pool.tile([128, DC * 512], bf16, name=f"xt_q{q}",
                                 tag="xt", bufs=4)
                xtq[q] = t
            eng.dma_start(
                t[:].rearrange("p (dc c) -> p dc c", c=512)[:, dc0:dc1, :],
                xt_d.ap()[:, q * 512:(q + 1) * 512]
                .rearrange("(dc p) c -> p dc c", p=128)[:, dc0:dc1, :])
            return t

        t0 = xt_quad(0, nc.sync, 0, 4)
        xt_quad(0, nc.scalar, 4, DC, tile_=t0)
        nc.sync.dma_start(wk_sb[0][:], wk_d[0].ap())
        nc.scalar.dma_start(wq_sb[0][:], wq_d[0].ap())
        nc.scalar.dma_start(cf32_sb[:], cf32_d.ap())
        # wvall split over both rings so dvv(0) (att_j(0,0) chunk 0, ~t+8us)
        # isn't stuck behind a 1MB transfer on one ring
        nc.sync.dma_start(wvall_sb[:, 0:DC * 128],
                          wvall_d.ap()[:, 0:DC * 128])
        nc.scalar.dma_start(wvall_sb[:, DC * 128:DC * 256],
                            wvall_d.ap()[:, DC * 128:DC * 256])
        nc.sync.dma_start(m01_sb[:], m01_d.ap())
        xt_quad(1, nc.sync)
        xt_quad(2, nc.sync)
        nc.sync.dma_start(wk_sb[1][:], wk_d[1].ap())
        nc.sync.dma_start(wq_sb[1][:], wq_d[1].ap())
        xt_quad(3, nc.sync)
        nc.sync.dma_start(wo_sb[0][:], wo_d[0].ap())
        nc.sync.dma_start(wo_sb[1][:], wo_d[1].ap())

        def xt_ap(dc, c0, c1):
            q = c0 // 512
            return xtq[q][:, dc * 512 + c0 - q * 512:dc * 512 + c1 - q * 512]

        # ---- per-pair tensors
        k2 = {p: kq_pool.tile([128, N], bf16, name=f"k2_p{p}", tag="kq")
              for p in range(NPAIR)}
        q2 = {p: kq_pool.tile([128, N], bf16, name=f"q2_p{p}", tag="kq")
              for p in range(NPAIR)}
        # v blocks per nb: [ones(64) | v_h0(64) | ones(64) | v_h1(64)] so the
        # PV lhsT for head h is the contiguous 128 cols at nb*256 + 128h and
        # psum rows 0:64 give the denominator for BOTH heads (base-0 rows:
        # reciprocal_approx_fast is a custom-DVE op and misreads partition-
        # offset operands on HW).
        v_sb = {}
        sa_sb = {}
        groups = {p: ((wk_sb[p], bk[p], k2[p]), (wq_sb[p], bq[p], q2[p]))
                  for p in range(NPAIR)}

        for p in range(NPAIR):
            v_sb[p] = v_pool.tile([128, NB * 256], bf16, name=f"v_p{p}",
                                  tag="v")
            sa_sb[p] = sa_pool.tile([128, N], bf16, name=f"sa_p{p}", tag="sa")
            nc.gpsimd.memset(
                v_sb[p][:].rearrange("p (nb h q) -> p nb h q", h=2, q=128)
                [:, :, :, 0:64], 1.0)

        def make_proj_slots(p, gi, nj):
            """One [128, 512] k/q projection group as two weave slots (dc 0-3
            and dc 4-7 + bias add), sharing the wv PSUM bank."""
            st = {}

            def part(d0, d1):
                def go():
                    if d0 == 0:
                        st["ps"] = wv_ps.tile([128, 512], f32, name="wvps",
                                              tag="wv")
                    ps = st["ps"]
                    w_sb, bias_ap, dst = groups[p][gi]
                    for dc in range(d0, d1):
                        nc.tensor.matmul(
                            ps[:], w_sb[:, dc * 128:(dc + 1) * 128],
                            xt_ap(dc, nj * 512, (nj + 1) * 512),
                            start=(dc == 0), stop=(dc == DC - 1))
                    if d1 == DC:
                        nc.vector.tensor_scalar_add(
                            dst[:, nj * 512:(nj + 1) * 512], ps[:], bias_ap)
                return go
            return [part(0, 4), part(4, DC)]

        def make_dvv(nb):
            """v projection for one 128-row block, all 4 heads in one
            xt-stationary matmul chain; strided DVE write adds the bias and
            drops the result into the [v|ones|v|ones] layout."""
            vps = tp_ps.tile([128, 256], f32, name="vps", tag="tp",
                             padded_shape=[128, 512])
            q, r = nb // 4, nb % 4
            for dc in range(DC):
                nc.tensor.matmul(vps[:],
                                 xtq[q][:, dc * 512 + r * 128:
                                         dc * 512 + (r + 1) * 128],
                                 wvall_sb[:, dc * 256:(dc + 1) * 256],
                                 start=(dc == 0), stop=(dc == DC - 1))
            for p in range(NPAIR):
                nc.vector.tensor_tensor(
                    v_sb[p][:, nb * 256:(nb + 1) * 256]
                    .rearrange("p (h q2) -> p h q2", q2=128)[:, :, 64:128],
                    vps[:, 128 * p:128 * p + 128]
                    .rearrange("p (h e) -> p h e", e=64),
                    bvr[p].rearrange("p (h e) -> p h e", e=64),
                    mybir.AluOpType.add)

        def make_op_slot(nb, dma_eng=None, cast_eng=None):
            """Output projection of one 128-row block: both pairs accumulate;
            the two column halves alternate over the wv / tp banks so casts
            overlap the next matmuls; one [128, 1024] bf16 DMA per block.
            Casts default to DVE; tail blocks use ACT (idle by then)."""
            def go():
                ob = ob_pool.tile([128, 1024], bf16, name="ob", tag="ob")
                for half in range(2):
                    pool, tag = (wv_ps, "wv") if half == 0 else (tp_ps, "tp")
                    op = pool.tile([128, 512], f32, name="opps", tag=tag,
                                   padded_shape=[128, 512])
                    for p in range(NPAIR):
                        nc.tensor.matmul(
                            op[:], sa_sb[p][:, nb * 128:(nb + 1) * 128],
                            wo_sb[p][:, half * 512:(half + 1) * 512],
                            start=(p == 0), stop=(p == NPAIR - 1))
                    ce = cast_eng
                    if ce == "mix":
                        ce = nc.vector if half == 0 else nc.scalar
                    if ce is nc.scalar:
                        nc.scalar.copy(ob[:, half * 512:(half + 1) * 512],
                                       op[:])
                    else:
                        nc.vector.tensor_copy(
                            ob[:, half * 512:(half + 1) * 512], op[:])
                (dma_eng or nc.sync).dma_start(
                    out_d.ap()[nb * 128:(nb + 1) * 128, :], ob[:])
            return go

        def kqv0_start():
            """Pair-A k/q projections for nj=0, d-chunk-major so the PE is
            paced by the q0 xt piece DMAs; all k matmuls first so the PE can
            start before wq0 lands (k/q in one score-pool tile)."""
            kq0s = s_ps.tile([128, 1024], f32, name="kq0s", tag="s")
            for dc in range(DC):
                nc.tensor.matmul(kq0s[:, 0:512],
                                 wk_sb[0][:, dc * 128:(dc + 1) * 128],
                                 xt_ap(dc, 0, 512),
                                 start=(dc == 0), stop=(dc == DC - 1))
            nc.vector.tensor_scalar_add(k2[0][:, 0:512], kq0s[:, 0:512],
                                        bk[0])
            for dc in range(DC):
                nc.tensor.matmul(kq0s[:, 512:1024],
                                 wq_sb[0][:, dc * 128:(dc + 1) * 128],
                                 xt_ap(dc, 0, 512),
                                 start=(dc == 0), stop=(dc == DC - 1))
            nc.vector.tensor_scalar_add(q2[0][:, 0:512], kq0s[:, 512:1024],
                                        bq[0])

        def emit_scores(p, j, ci, state, mask_eng=None):
            """Scores (both heads into one 2-bank psum tile) + exp for one
            128-m chunk."""
            t = ci - 4 * j
            lo = 128 * t if t >= 0 else 0
            sp = s_ps.tile([128, 1024], f32, name="s", tag="s")
            pt = pt_pool.tile([128, 1024], bf16, name="pt", tag="pt")
            for h in range(HL):
                hp = 64 * h
                nc.tensor.matmul(
                    sp[:, 512 * h + lo:512 * h + 512],
                    q2[p][hp:hp + 64, ci * 128:(ci + 1) * 128],
                    k2[p][hp:hp + 64, j * 512 + lo:(j + 1) * 512],
                    start=True, stop=True)
            if t < 0:
                nc.scalar.activation(pt[:], sp[:], Exp, scale=0.125)
            else:
                # one strided activation covering both heads' valid region
                nc.scalar.activation(
                    pt[:].rearrange("p (h w) -> p h w", h=HL)[:, :, lo:512],
                    sp[:].rearrange("p (h w) -> p h w", h=HL)[:, :, lo:512],
                    Exp, scale=0.125)
                for h in range(HL):
                    # bf16 in/out: cheap on DVE; the last block's masks go to
                    # Pool instead so they don't queue behind the previous
                    # block's norm chain on DVE (bf16-only Pool ops are fine)
                    (mask_eng or nc.vector).tensor_tensor(
                        pt[:, 512 * h + lo:512 * h + lo + 128],
                        pt[:, 512 * h + lo:512 * h + lo + 128],
                        m01_sb[:], mybir.AluOpType.mult)
            state[ci] = (pt, lo)

        def emit_pv(p, j, ci, pv2, state, nch):
            pt, lo = state.pop(ci)
            for h in range(HL):
                nc.tensor.matmul(
                    pv2[:, 512 * h + lo:512 * (h + 1)],
                    v_sb[p][:, ci * 256 + 128 * h:ci * 256 + 128 * h + 128],
                    pt[:, 512 * h + lo:512 * h + 512],
                    start=(ci == 0), stop=(ci == nch - 1))

        def att_norm(p, j, pv2, last=False):
            # psum rows 0:64 = denominator, 64:128 = sa, for BOTH heads.
            # ONE full-tile copy (DVE cost is free-size only, so copying all
            # 128 rows costs the same as 64) frees the PV psum in ~1.25us so
            # the next block's PV isn't WAR-blocked; the mults become
            # SBUF-only and run on the Pool engine, keeping the DVE queue
            # short ahead of the next block's diag masks.
            # denominators are sums of exp() in [~2e-3, ~3e3]: inside
            # approx_fast's domain; 18-bit accuracy is far below the bf16
            # noise of the P*V numerator. (approx_fast misreads PSUM
            # operands on HW - bounce through SBUF first.)
            den_sb = rc_pool.tile([64, 1024], f32, name="den", tag="den")
            nc.vector.tensor_copy(den_sb[:], pv2[0:64, :])
            if last:
                # no successor uses the pv psum bank: read sa straight from
                # PSUM (exempt from the same-start-partition rule), saving
                # one copy on the tail critical path
                saf = pv2[64:128, :]
            else:
                saf_t = rc_pool.tile([64, 1024], f32, name="saf", tag="saf")
                nc.vector.tensor_copy(saf_t[:], pv2[64:128, :])
                saf = saf_t[:]
            rc = rc_pool.tile([64, 1024], f32, name="rc", tag="rc")
            nc.vector.reciprocal_approx_fast(rc[:], den_sb[:])
            for h in range(HL):
                # fp32-in/bf16-out tensor_tensor gives wrong results on the
                # Pool engine on HW (sim is fine) - keep these on DVE; all
                # SBUF inputs are base-partition-0 (verifier requirement,
                # and custom-DVE ops misread partition-offset operands)
                nc.vector.tensor_tensor(
                    sa_sb[p][64 * h:64 * h + 64, j * 512:(j + 1) * 512],
                    saf[:, 512 * h:512 * (h + 1)],
                    rc[:, 512 * h:512 * (h + 1)], mybir.AluOpType.mult)

        def att_j(p, j, weave=(), dv=False, wd=0, late=None, defer_norm=False,
                  mask_eng=None, last=False):
            """One attention j-block, software-pipelined: scores/exp of chunk
            ci+1 are emitted before PV of chunk ci; weave slots fill PE gaps
            (starting at chunk `wd` — op-slot weaves depend on the previous
            block's att_norm, so give that DVE chain a head start); in pair-0
            blocks the v projections for this j's rows are emitted inline
            where the PE would wait for exp. `late` (e.g. the previous
            block's deferred norm) is emitted after chunk 1 so this block's
            first diag masks aren't queued behind it on DVE. With
            `defer_norm` the caller gets a closure to emit this block's norm
            later."""
            pv2 = pv_ps.tile([128, 1024], f32, name="pv2", tag="pv")
            weave = list(weave)
            nch = 4 * (j + 1)
            state = {}
            emitted = 0
            emit_scores(p, j, 0, state, mask_eng=mask_eng)
            for ci in range(nch):
                if ci + 1 < nch:
                    emit_scores(p, j, ci + 1, state, mask_eng=mask_eng)
                if ci == 1 and late is not None:
                    late()
                if ci >= wd:
                    target = len(weave) * (ci + 1 - wd) // (nch - wd)
                    while emitted < target:
                        weave[emitted]()
                        emitted += 1
                if dv and ci < 4:
                    make_dvv(4 * j + ci)
                emit_pv(p, j, ci, pv2, state, nch)
            # norm first so its DVE chain heads the queue; leftover weave
            # slots (PE/ACT work) then overlap it
            if defer_norm:
                norm = [lambda: att_norm(p, j, pv2, last=last)]
            else:
                att_norm(p, j, pv2, last=last)
                norm = []
            while emitted < len(weave):
                weave[emitted]()
                emitted += 1
            return norm[0] if norm else None

        # ================= emission schedule =================
        def ksl(p, nj):
            return make_proj_slots(p, 0, nj)

        def qsl(p, nj):
            return make_proj_slots(p, 1, nj)

        def ops(nbs, cast_eng=None):
            return [make_op_slot(nb, cast_eng=cast_eng) for nb in nbs]

        kqv0_start()
        att_j(0, 0, weave=ksl(0, 1) + qsl(0, 1), dv=True)
        att_j(0, 1, weave=ksl(0, 2) + qsl(0, 2), dv=True)
        # (1,3)'s scores read ALL of q2[1], so every qsl(1,*) must precede
        # it; only the k-blocks of later pair-1 js may ride inside it
        att_j(0, 2, weave=ksl(0, 3) + qsl(0, 3) + qsl(1, 0) + qsl(1, 1),
              dv=True)
        att_j(0, 3, weave=qsl(1, 2) + qsl(1, 3) + ksl(1, 3) + ksl(1, 2),
              dv=True)
        # pair-1 j-blocks DESCEND so the last block (and tail) is small;
        # ops() weaves start wd chunks in: the previous block's att_norm
        # (whose sa they read) is a ~5.6us DVE chain behind the last diag
        # masks, so firing an op matmul earlier stalls the in-order PE queue
        att_j(1, 3, weave=ksl(1, 1) + ksl(1, 0))
        att_j(1, 2, weave=ops(range(12, 16)), wd=5)
        att_j(1, 1, weave=ops(range(8, 12)), wd=6)
        # wd=4 > nch pushes the whole weave to the post-norm flush: op(4..7)
        # matmuls (ACT casts) overlap norm(1,0) on DVE
        att_j(1, 0, weave=ops(range(4, 8), cast_eng=nc.scalar), wd=4,
              mask_eng=nc.gpsimd)
        # tail: alternate the last out-DMAs over the two HW-DGE queues and
        # split casts across DVE/ACT so neither serializes the drain
        for nb, eng in zip(range(0, 4),
                           (nc.sync, nc.scalar, nc.sync, nc.scalar)):
            make_op_slot(nb, dma_eng=eng, cast_eng="mix")()

    nc.compile()
    return nc


def _get_nc():
    if "nc" not in _CACHE:
        _CACHE["nc"] = _build()
    return _CACHE["nc"]


def _prep_inputs(x, Wkqv, bkqv, Wo, bo):
    """Host-side shard prep: one input map per core (core c: batch c//4,
    head-pairs (4*(c%4), 4*(c%4)+1) and (+2, +3))."""
    tri = np.triu(np.ones((128, 128), np.float32)).astype(BF16)  # m' <= n''
    xts = [np.ascontiguousarray(x[b].T).astype(BF16) for b in range(B)]

    def shuf(w):
        # [D, 128] -> [128, DC*128]: partition = within-chunk row
        return np.ascontiguousarray(
            w.reshape(DC, 128, 128).transpose(1, 0, 2).reshape(128, DC * 128))

    in_maps = []
    for c in range(NCORES):
        b, m = c // 4, c % 4
        im = {"xt": xts[b], "m01": tri}
        wvs = {}
        cf32 = np.zeros((128, 260), np.float32)
        for p in range(NPAIR):
            h0 = 4 * m + 2 * p
            h1 = h0 + 1
            im[f"wk{p}"] = shuf(np.concatenate(
                [Wkqv[h0, :, 0:64], Wkqv[h1, :, 0:64]], axis=1)).astype(BF16)
            im[f"wq{p}"] = shuf(np.concatenate(
                [Wkqv[h0, :, 64:128], Wkqv[h1, :, 64:128]], axis=1)).astype(BF16)
            wvs[p] = shuf(np.concatenate(
                [Wkqv[h0, :, 128:192], Wkqv[h1, :, 128:192]], axis=1)).astype(BF16)
            im[f"wo{p}"] = Wo[64 * h0:64 * h0 + 128, :].astype(BF16)
            bv_pair = np.concatenate([bkqv[h0, 128:192], bkqv[h1, 128:192]])
            cf32[:, 128 * p:128 * p + 128] = bv_pair[None, :]
            cf32[:, 256 + 2 * p] = np.concatenate(
                [bkqv[h0, 0:64], bkqv[h1, 0:64]])
            cf32[:, 257 + 2 * p] = np.concatenate(
                [bkqv[h0, 64:128], bkqv[h1, 64:128]])
        # combined v weights: per chunk dc, 256 cols = [pair0 128 | pair1 128]
        wvall = np.zeros((128, DC * 256), np.float32).astype(BF16)
        for dc in range(DC):
            wvall[:, dc * 256:dc * 256 + 128] = wvs[0][:, dc * 128:(dc + 1) * 128]
            wvall[:, dc * 256 + 128:dc * 256 + 256] = wvs[1][:, dc * 128:(dc + 1) * 128]
        im["wvall"] = wvall
        im["cf32"] = cf32
        in_maps.append(im)
    return in_maps


def kernel(x, Wkqv, bkqv, Wo, bo):
    from concourse import bass_utils

    nc = _get_nc()
    in_maps = _prep_inputs(np.asarray(x), np.asarray(Wkqv), np.asarray(bkqv),
                           np.asarray(Wo), np.asarray(bo))
    res = bass_utils.run_bass_kernel_spmd(nc, in_maps, core_ids=list(range(NCORES)))
    acc = np.zeros((B, N, D), np.float32)
    for c in range(NCORES):
        acc[c // 4] += np.asarray(res.results[c]["out"], dtype=np.float32)
    acc += np.asarray(bo)[None, None, :]
    return acc


# revision 60
# speedup vs baseline: 1.0341x; 1.0054x over previous
"""Causal self-attention (k/q swapped variant) on 8 Trainium2 NeuronCores.

Problem (hardcoded shapes): B=2, N=2048, D=1024, H=16, DH=64.
  kqv = einsum('bnd,hde->bhne', x, Wkqv) + bkqv   ; split -> k, q, v
  A[b,h,n,m] = k[b,h,n]·q[b,h,m] / sqrt(DH), causal mask m<=n, softmax over m
  sa = A @ v ; concat heads ; out = sa @ Wo + bo

Sharding: batch x heads — core c owns batch c//4 and heads 4*(c%4)..+4 (two
head-pairs A/B), computes its partial output projection sa_local @ Wo[rows]
in bf16 over its single batch, and the host sums 4 partials per batch (+bo)
in fp32. This halves both the x input DMA and the partial-output DMA vs
all-batches-per-core head sharding.

Per-core device kernel (all matmul operands bf16, fp32 PSUM accumulation):
  - x is pre-transposed on host to xt = x[b].T ([D, N]) so the contraction
    dim d lands on SBUF partitions; one copy shared by both head-pairs.
  - k/q are projected with W-stationary matmuls into [dh, n] layout; v is
    projected directly into [n, dh] layout with xt-stationary matmuls (one
    matmul covers all 4 heads), so no PE transposes / extra copies at all.
  - scores are computed transposed, S^T[m, n] = q[m]·k[n], so softmax's
    reduction dim m sits on partitions; both heads of a pair live in ONE
    [128, 1024] PSUM tile (2 banks) so off-diagonal chunks need a single wide
    exp() on the ACT engine (diag chunks use one strided-AP exp). The
    denominator comes free from the PV matmul: v blocks are [v_h|ones] so
    PV psum rows 64:128 hold the softmax denominators for both heads.
  - the chunk loop is software-pipelined: scores+exp of chunk ci+1 are
    emitted before the PV of chunk ci; k/q projection and output-projection
    slots are woven between chunks; v-block projections are emitted inline
    in pair-0 blocks right where the PE would otherwise wait for exp.
  - pair-1 attention blocks run in DESCENDING j order so the final block is
    the 4-chunk j=0 and the tail (last norm + out-blocks 0..3) is short.
  - PSUM budget (8 banks): scores 2x[128,1024] (4) + PV accumulator
    [128,1024] (2) + proj/outproj bank 1 + v-proj/outproj bank 1.
"""

import numpy as np
import ml_dtypes

B = 2
N = 2048
D = 1024
H = 16
DH = 64
NCORES = 8
HL = 2                    # heads per pair
NPAIR = 2                 # head-pairs per core
DC = D // 128             # contraction chunks = 8
NB = N // 128             # 128-row blocks = 16
NJ = N // 512             # 512-col blocks = 4

BF16 = ml_dtypes.bfloat16

_CACHE = {}


def _build():
    import concourse.bass as bass
    import concourse.mybir as mybir
    import concourse.tile as tile
    from concourse import bacc
    from contextlib import ExitStack

    f32 = mybir.dt.float32
    bf16 = mybir.dt.bfloat16
    Exp = mybir.ActivationFunctionType.Exp

    nc = bacc.Bacc("TRN2", target_bir_lowering=False, debug=False,
                   enable_asserts=False, num_devices=NCORES)

    xt_d = nc.dram_tensor("xt", [D, N], bf16, kind="ExternalInput")
    # k/q weights arrive pre-shuffled to the SBUF layout [128, DC*128]
    # (partition = within-chunk row, free = (chunk, head-col)), per pair
    wk_d = {p: nc.dram_tensor(f"wk{p}", [128, DC * 128], bf16,
                              kind="ExternalInput") for p in range(NPAIR)}
    wq_d = {p: nc.dram_tensor(f"wq{p}", [128, DC * 128], bf16,
                              kind="ExternalInput") for p in range(NPAIR)}
    # v weights combined for both pairs: per chunk dc the 256 cols are
    # [pair0 h0|h1 | pair1 h0|h1]
    wvall_d = nc.dram_tensor("wvall", [128, DC * 256], bf16,
                             kind="ExternalInput")
    wo_d = {p: nc.dram_tensor(f"wo{p}", [128, D], bf16,
                              kind="ExternalInput") for p in range(NPAIR)}
    # packed f32 consts: [bvr0(128) | bvr1(128) | bk0 | bq0 | bk1 | bq1]
    cf32_d = nc.dram_tensor("cf32", [128, 260], f32, kind="ExternalInput")
    m01_d = nc.dram_tensor("m01", [128, 128], bf16, kind="ExternalInput")
    out_d = nc.dram_tensor("out", [N, D], bf16, kind="ExternalOutput")

    with tile.TileContext(nc) as tc, ExitStack() as ctx:
        const = ctx.enter_context(tc.tile_pool(name="const", bufs=1))
        xt_pool = ctx.enter_context(tc.tile_pool(name="xt", bufs=1))
        kq_pool = ctx.enter_context(tc.tile_pool(name="kq", bufs=4))
        v_pool = ctx.enter_context(tc.tile_pool(name="v", bufs=2))
        sa_pool = ctx.enter_context(tc.tile_pool(name="sa", bufs=2))
        pt_pool = ctx.enter_context(tc.tile_pool(name="pt", bufs=6))
        rc_pool = ctx.enter_context(tc.tile_pool(name="rc", bufs=2))
        ob_pool = ctx.enter_context(tc.tile_pool(name="ob", bufs=4))
        s_ps = ctx.enter_context(tc.tile_pool(name="s_ps", bufs=2, space="PSUM"))
        pv_ps = ctx.enter_context(tc.tile_pool(name="pv_ps", bufs=1, space="PSUM"))
        wv_ps = ctx.enter_context(tc.tile_pool(name="wv_ps", bufs=1, space="PSUM"))
        tp_ps = ctx.enter_context(tc.tile_pool(name="tp_ps", bufs=1, space="PSUM"))

        wk_sb, wq_sb, wo_sb = {}, {}, {}
        for p in range(NPAIR):
            wk_sb[p] = const.tile([128, DC * 128], bf16, name=f"wk_sb{p}")
            wq_sb[p] = const.tile([128, DC * 128], bf16, name=f"wq_sb{p}")
            wo_sb[p] = const.tile([128, D], bf16, name=f"wo_sb{p}")
        wvall_sb = const.tile([128, DC * 256], bf16, name="wvall_sb")
        cf32_sb = const.tile([128, 260], f32, name="cf32_sb")
        m01_sb = const.tile([128, 128], bf16, name="m01_sb")
        bvr = {p: cf32_sb[:, 128 * p:128 * p + 128] for p in range(NPAIR)}
        bk = {p: cf32_sb[:, 256 + 2 * p:257 + 2 * p] for p in range(NPAIR)}
        bq = {p: cf32_sb[:, 257 + 2 * p:258 + 2 * p] for p in range(NPAIR)}

        # ---- DMA issue order: what the start of the pipeline needs first.
        # kqv0 needs wk0 + xt q0; the first dvv needs wvall + bvr; the first
        # diag masks need m01; q1 feeds the first weave slots. Each xt
        # quadrant is ONE multi-dim DMA ([p, dc, c] pattern) — per-DMA ring
        # time (~0.6us) dominates piece transfers, so fewer, bigger is
        # faster. q0 is split in two (dc 0-3 / 4-7 over both rings) so the
        # kqv0 chain can start sooner.
        xtq = {}   # quad -> [128, DC*512] tile (chunk dc at cols dc*512)

        def xt_quad(q, eng, dc0=0, dc1=DC, tile_=None):
            t = tile_
            if t is None:
                t = xt_pool.tile([128, DC * 512], bf16, name=f"xt_q{q}",
                                 tag="xt", bufs=4)
                xtq[q] = t
            eng.dma_start(
                t[:].rearrange("p (dc c) -> p dc c", c=512)[:, dc0:dc1, :],
                xt_d.ap()[:, q * 512:(q + 1) * 512]
                .rearrange("(dc p) c -> p dc c", p=128)[:, dc0:dc1, :])
            return t

        t0 = xt_quad(0, nc.sync, 0, 4)
        xt_quad(0, nc.scalar, 4, DC, tile_=t0)
        nc.sync.dma_start(wk_sb[0][:], wk_d[0].ap())
        nc.scalar.dma_start(wq_sb[0][:], wq_d[0].ap())
        nc.scalar.dma_start(cf32_sb[:], cf32_d.ap())
        # wvall split over both rings so dvv(0) (att_j(0,0) chunk 0, ~t+8us)
        # isn't stuck behind a 1MB transfer on one ring
        nc.sync.dma_start(wvall_sb[:, 0:DC * 128],
                          wvall_d.ap()[:, 0:DC * 128])
        nc.scalar.dma_start(wvall_sb[:, DC * 128:DC * 256],
                            wvall_d.ap()[:, DC * 128:DC * 256])
        nc.sync.dma_start(m01_sb[:], m01_d.ap())
        xt_quad(1, nc.sync)
        xt_quad(2, nc.sync)
        nc.sync.dma_start(wk_sb[1][:], wk_d[1].ap())
        nc.sync.dma_start(wq_sb[1][:], wq_d[1].ap())
        xt_quad(3, nc.sync)
        nc.sync.dma_start(wo_sb[0][:], wo_d[0].ap())
        nc.sync.dma_start(wo_sb[1][:], wo_d[1].ap())

        def xt_ap(dc, c0, c1):
            q = c0 // 512
            return xtq[q][:, dc * 512 + c0 - q * 512:dc * 512 + c1 - q * 512]

        # ---- per-pair tensors
        k2 = {p: kq_pool.tile([128, N], bf16, name=f"k2_p{p}", tag="kq")
              for p in range(NPAIR)}
        q2 = {p: kq_pool.tile([128, N], bf16, name=f"q2_p{p}", tag="kq")
              for p in range(NPAIR)}
        # v blocks per nb: [ones(64) | v_h0(64) | ones(64) | v_h1(64)] so the
        # PV lhsT for head h is the contiguous 128 cols at nb*256 + 128h and
        # psum rows 0:64 give the denominator for BOTH heads (base-0 rows:
        # reciprocal_approx_fast is a custom-DVE op and misreads partition-
        # offset operands on HW).
        v_sb = {}
        sa_sb = {}
        groups = {p: ((wk_sb[p], bk[p], k2[p]), (wq_sb[p], bq[p], q2[p]))
                  for p in range(NPAIR)}

        for p in range(NPAIR):
            v_sb[p] = v_pool.tile([128, NB * 256], bf16, name=f"v_p{p}",
                                  tag="v")
            sa_sb[p] = sa_pool.tile([128, N], bf16, name=f"sa_p{p}", tag="sa")
            nc.gpsimd.memset(
                v_sb[p][:].rearrange("p (nb h q) -> p nb h q", h=2, q=128)
                [:, :, :, 0:64], 1.0)

        def make_proj_slots(p, gi, nj):
            """One [128, 512] k/q projection group as two weave slots (dc 0-3
            and dc 4-7 + bias add), sharing the wv PSUM bank."""
            st = {}

            def part(d0, d1):
                def go():
                    if d0 == 0:
                        st["ps"] = wv_ps.tile([128, 512], f32, name="wvps",
                                              tag="wv")
                    ps = st["ps"]
                    w_sb, bias_ap, dst = groups[p][gi]
                    for dc in range(d0, d1):
                        nc.tensor.matmul(
                            ps[:], w_sb[:, dc * 128:(dc + 1) * 128],
                            xt_ap(dc, nj * 512, (nj + 1) * 512),
                            start=(dc == 0), stop=(dc == DC - 1))
                    if d1 == DC:
                        nc.vector.tensor_scalar_add(
                            dst[:, nj * 512:(nj + 1) * 512], ps[:], bias_ap)
                return go
            return [part(0, 4), part(4, DC)]

        def make_dvv(nb):
            """v projection for one 128-row block, all 4 heads in one
            xt-stationary matmul chain; strided DVE write adds the bias and
            drops the result into the [v|ones|v|ones] layout."""
            vps = tp_ps.tile([128, 256], f32, name="vps", tag="tp",
                             padded_shape=[128, 512])
            q, r = nb // 4, nb % 4
            for dc in range(DC):
                nc.tensor.matmul(vps[:],
                                 xtq[q][:, dc * 512 + r * 128:
                                         dc * 512 + (r + 1) * 128],
                                 wvall_sb[:, dc * 256:(dc + 1) * 256],
                                 start=(dc == 0), stop=(dc == DC - 1))
            for p in range(NPAIR):
                nc.vector.tensor_tensor(
                    v_sb[p][:, nb * 256:(nb + 1) * 256]
                    .rearrange("p (h q2) -> p h q2", q2=128)[:, :, 64:128],
                    vps[:, 128 * p:128 * p + 128]
                    .rearrange("p (h e) -> p h e", e=64),
                    bvr[p].rearrange("p (h e) -> p h e", e=64),
                    mybir.AluOpType.add)

        def make_op_slot(nb, dma_eng=None, cast_eng=None):
            """Output projection of one 128-row block: both pairs accumulate;
            the two column halves alternate over the wv / tp banks so casts
            overlap the next matmuls; one [128, 1024] bf16 DMA per block.
            Casts default to DVE; tail blocks use ACT (idle by then)."""
            def go():
                ob = ob_pool.tile([128, 1024], bf16, name="ob", tag="ob")
                for half in range(2):
                    pool, tag = (wv_ps, "wv") if half == 0 else (tp_ps, "tp")
                    op = pool.tile([128, 512], f32, name="opps", tag=tag,
                                   padded_shape=[128, 512])
                    for p in range(NPAIR):
                        nc.tensor.matmul(
                            op[:], sa_sb[p][:, nb * 128:(nb + 1) * 128],
                            wo_sb[p][:, half * 512:(half + 1) * 512],
                            start=(p == 0), stop=(p == NPAIR - 1))
                    ce = cast_eng
                    if ce == "mix":
                        ce = nc.vector if half == 0 else nc.scalar
                    if ce is nc.scalar:
                        nc.scalar.copy(ob[:, half * 512:(half + 1) * 512],
                                       op[:])
                    else:
                        nc.vector.tensor_copy(
                            ob[:, half * 512:(half + 1) * 512], op[:])
                (dma_eng or nc.sync).dma_start(
                    out_d.ap()[nb * 128:(nb + 1) * 128, :], ob[:])
            return go

        def kqv0_start():
            """Pair-A k/q projections for nj=0, d-chunk-major so the PE is
            paced by the q0 xt piece DMAs; all k matmuls first so the PE can
            start before wq0 lands (k/q in one score-pool tile)."""
            kq0s = s_ps.tile([128, 1024], f32, name="kq0s", tag="s")
            for dc in range(DC):
                nc.tensor.matmul(kq0s[:, 0:512],
                                 wk_sb[0][:, dc * 128:(dc + 1) * 128],
                                 xt_ap(dc, 0, 512),
                                 start=(dc == 0), stop=(dc == DC - 1))
            nc.vector.tensor_scalar_add(k2[0][:, 0:512], kq0s[:, 0:512],
                                        bk[0])
            for dc in range(DC):
                nc.tensor.matmul(kq0s[:, 512:1024],
                                 wq_sb[0][:, dc * 128:(dc + 1) * 128],
                                 xt_ap(dc, 0, 512),
                                 start=(dc == 0), stop=(dc == DC - 1))
            nc.vector.tensor_scalar_add(q2[0][:, 0:512], kq0s[:, 512:1024],
                                        bq[0])

        def emit_scores(p, j, ci, state, mask_eng=None):
            """Scores (both heads into one 2-bank psum tile) + exp for one
            128-m chunk."""
            t = ci - 4 * j
            lo = 128 * t if t >= 0 else 0
            sp = s_ps.tile([128, 1024], f32, name="s", tag="s")
            pt = pt_pool.tile([128, 1024], bf16, name="pt", tag="pt")
            for h in range(HL):
                hp = 64 * h
                nc.tensor.matmul(
                    sp[:, 512 * h + lo:512 * h + 512],
                    q2[p][hp:hp + 64, ci * 128:(ci + 1) * 128],
                    k2[p][hp:hp + 64, j * 512 + lo:(j + 1) * 512],
                    start=True, stop=True)
            if t < 0:
                nc.scalar.activation(pt[:], sp[:], Exp, scale=0.125)
            else:
                # one strided activation covering both heads' valid region
                nc.scalar.activation(
                    pt[:].rearrange("p (h w) -> p h w", h=HL)[:, :, lo:512],
                    sp[:].rearrange("p (h w) -> p h w", h=HL)[:, :, lo:512],
                    Exp, scale=0.125)
                for h in range(HL):
                    # bf16 in/out: cheap on DVE; the last block's masks go to
                    # Pool instead so they don't queue behind the previous
                    # block's norm chain on DVE (bf16-only Pool ops are fine)
                    (mask_eng or nc.vector).tensor_tensor(
                        pt[:, 512 * h + lo:512 * h + lo + 128],
                        pt[:, 512 * h + lo:512 * h + lo + 128],
                        m01_sb[:], mybir.AluOpType.mult)
            state[ci] = (pt, lo)

        def emit_pv(p, j, ci, pv2, state, nch):
            pt, lo = state.pop(ci)
            for h in range(HL):
                nc.tensor.matmul(
                    pv2[:, 512 * h + lo:512 * (h + 1)],
                    v_sb[p][:, ci * 256 + 128 * h:ci * 256 + 128 * h + 128],
                    pt[:, 512 * h + lo:512 * h + 512],
                    start=(ci == 0), stop=(ci == nch - 1))

        def att_norm(p, j, pv2, last=False):
            # psum rows 0:64 = denominator, 64:128 = sa, for BOTH heads.
            # ONE full-tile copy (DVE cost is free-size only, so copying all
            # 128 rows costs the same as 64) frees the PV psum in ~1.25us so
            # the next block's PV isn't WAR-blocked; the mults become
            # SBUF-only and run on the Pool engine, keeping the DVE queue
            # short ahead of the next block's diag masks.
            # denominators are sums of exp() in [~2e-3, ~3e3]: inside
            # approx_fast's domain; 18-bit accuracy is far below the bf16
            # noise of the P*V numerator. (approx_fast misreads PSUM
            # operands on HW - bounce through SBUF first.)
            den_sb = rc_pool.tile([64, 1024], f32, name="den", tag="den")
            nc.vector.tensor_copy(den_sb[:], pv2[0:64, :])
            if last:
                # no successor uses the pv psum bank: read sa straight from
                # PSUM (exempt from the same-start-partition rule), saving
                # one copy on the tail critical path
                saf = pv2[64:128, :]
            else:
                saf_t = rc_pool.tile([64, 1024], f32, name="saf", tag="saf")
                nc.vector.tensor_copy(saf_t[:], pv2[64:128, :])
                saf = saf_t[:]
            rc = rc_pool.tile([64, 1024], f32, name="rc", tag="rc")
            nc.vector.reciprocal_approx_fast(rc[:], den_sb[:])
            for h in range(HL):
                # fp32-in/bf16-out tensor_tensor gives wrong results on the
                # Pool engine on HW (sim is fine) - keep these on DVE; all
                # SBUF inputs are base-partition-0 (verifier requirement,
                # and custom-DVE ops misread partition-offset operands)
                nc.vector.tensor_tensor(
                    sa_sb[p][64 * h:64 * h + 64, j * 512:(j + 1) * 512],
                    saf[:, 512 * h:512 * (h + 1)],
                    rc[:, 512 * h:512 * (h + 1)], mybir.AluOpType.mult)

        def att_j(p, j, weave=(), dv=False, wd=0, late=None, defer_norm=False,
                  mask_eng=None, last=False):
            """One attention j-block, software-pipelined: scores/exp of chunk
            ci+1 are emitted before PV of chunk ci; weave slots fill PE gaps
            (starting at chunk `wd` — op-slot weaves depend on the previous
            block's att_norm, so give that DVE chain a head start); in pair-0
            blocks the v projections for this j's rows are emitted inline
            where the PE would wait for exp. `late` (e.g. the previous
            block's deferred norm) is emitted after chunk 1 so this block's
            first diag masks aren't queued behind it on DVE. With
            `defer_norm` the caller gets a closure to emit this block's norm
            later."""
            pv2 = pv_ps.tile([128, 1024], f32, name="pv2", tag="pv")
            weave = list(weave)
            nch = 4 * (j + 1)
            state = {}
            emitted = 0
            emit_scores(p, j, 0, state, mask_eng=mask_eng)
            for ci in range(nch):
                if ci + 1 < nch:
                    emit_scores(p, j, ci + 1, state, mask_eng=mask_eng)
                if ci == 1 and late is not None:
                    late()
                if ci >= wd:
                    target = len(weave) * (ci + 1 - wd) // (nch - wd)
                    while emitted < target:
                        weave[emitted]()
                        emitted += 1
                if dv and ci < 4:
                    make_dvv(4 * j + ci)
                emit_pv(p, j, ci, pv2, state, nch)
            # norm first so its DVE chain heads the queue; leftover weave
            # slots (PE/ACT work) then overlap it
            if defer_norm:
                norm = [lambda: att_norm(p, j, pv2, last=last)]
            else:
                att_norm(p, j, pv2, last=last)
                norm = []
            while emitted < len(weave):
                weave[emitted]()
                emitted += 1
            return norm[0] if norm else None

        # ================= emission schedule =================
        def ksl(p, nj):
            return make_proj_slots(p, 0, nj)

        def qsl(p, nj):
            return make_proj_slots(p, 1, nj)

        def ops(nbs, cast_eng=None):
            return [make_op_slot(nb, cast_eng=cast_eng) for nb in nbs]

        kqv0_start()
        att_j(0, 0, weave=ksl(0, 1) + qsl(0, 1), dv=True)
        att_j(0, 1, weave=ksl(0, 2) + qsl(0, 2), dv=True)
        # (1,3)'s scores read ALL of q2[1], so every qsl(1,*) must precede
        # it; only the k-blocks of later pair-1 js may ride inside it
        att_j(0, 2, weave=ksl(0, 3) + qsl(0, 3) + qsl(1, 0) + qsl(1, 1),
              dv=True)
        att_j(0, 3, weave=qsl(1, 2) + qsl(1, 3) + ksl(1, 3) + ksl(1, 2),
              dv=True)
        # pair-1 j-blocks DESCEND so the last block (and tail) is small;
        # ops() weaves start wd chunks in: the previous block's att_norm
        # (whose sa they read) is a ~5.6us DVE chain behind the last diag
        # masks, so firing an op matmul earlier stalls the in-order PE queue
        att_j(1, 3, weave=ksl(1, 1) + ksl(1, 0))
        att_j(1, 2, weave=ops(range(12, 16)), wd=5)
        att_j(1, 1, weave=ops(range(8, 12)), wd=6)
        # wd=4 > nch pushes the whole weave to the post-norm flush: op(4..7)
        # matmuls (ACT casts) overlap norm(1,0) on DVE
        att_j(1, 0, weave=ops(range(4, 8), cast_eng=nc.scalar), wd=4,
              mask_eng=nc.gpsimd, last=True)
        # tail: alternate the last out-DMAs over the two HW-DGE queues and
        # split casts across DVE/ACT so neither serializes the drain
        for nb, eng in zip(range(0, 4),
                           (nc.sync, nc.scalar, nc.sync, nc.scalar)):
            make_op_slot(nb, dma_eng=eng, cast_eng="mix")()

    nc.compile()
    return nc


def _get_nc():
    if "nc" not in _CACHE:
        _CACHE["nc"] = _build()
    return _CACHE["nc"]


def _prep_inputs(x, Wkqv, bkqv, Wo, bo):
    """Host-side shard prep: one input map per core (core c: batch c//4,
    head-pairs (4*(c%4), 4*(c%4)+1) and (+2, +3))."""
    tri = np.triu(np.ones((128, 128), np.float32)).astype(BF16)  # m' <= n''
    xts = [np.ascontiguousarray(x[b].T).astype(BF16) for b in range(B)]

    def shuf(w):
        # [D, 128] -> [128, DC*128]: partition = within-chunk row
        return np.ascontiguousarray(
            w.reshape(DC, 128, 128).transpose(1, 0, 2).reshape(128, DC * 128))

    in_maps = []
    for c in range(NCORES):
        b, m = c // 4, c % 4
        im = {"xt": xts[b], "m01": tri}
        wvs = {}
        cf32 = np.zeros((128, 260), np.float32)
        for p in range(NPAIR):
            h0 = 4 * m + 2 * p
            h1 = h0 + 1
            im[f"wk{p}"] = shuf(np.concatenate(
                [Wkqv[h0, :, 0:64], Wkqv[h1, :, 0:64]], axis=1)).astype(BF16)
            im[f"wq{p}"] = shuf(np.concatenate(
                [Wkqv[h0, :, 64:128], Wkqv[h1, :, 64:128]], axis=1)).astype(BF16)
            wvs[p] = shuf(np.concatenate(
                [Wkqv[h0, :, 128:192], Wkqv[h1, :, 128:192]], axis=1)).astype(BF16)
            im[f"wo{p}"] = Wo[64 * h0:64 * h0 + 128, :].astype(BF16)
            bv_pair = np.concatenate([bkqv[h0, 128:192], bkqv[h1, 128:192]])
            cf32[:, 128 * p:128 * p + 128] = bv_pair[None, :]
            cf32[:, 256 + 2 * p] = np.concatenate(
                [bkqv[h0, 0:64], bkqv[h1, 0:64]])
            cf32[:, 257 + 2 * p] = np.concatenate(
                [bkqv[h0, 64:128], bkqv[h1, 64:128]])
        # combined v weights: per chunk dc, 256 cols = [pair0 128 | pair1 128]
        wvall = np.zeros((128, DC * 256), np.float32).astype(BF16)
        for dc in range(DC):
            wvall[:, dc * 256:dc * 256 + 128] = wvs[0][:, dc * 128:(dc + 1) * 128]
            wvall[:, dc * 256 + 128:dc * 256 + 256] = wvs[1][:, dc * 128:(dc + 1) * 128]
        im["wvall"] = wvall
        im["cf32"] = cf32
        in_maps.append(im)
    return in_maps


def kernel(x, Wkqv, bkqv, Wo, bo):
    from concourse import bass_utils

    nc = _get_nc()
    in_maps = _prep_inputs(np.asarray(x), np.asarray(Wkqv), np.asarray(bkqv),
                           np.asarray(Wo), np.asarray(bo))
    res = bass_utils.run_bass_kernel_spmd(nc, in_maps, core_ids=list(range(NCORES)))
    acc = np.zeros((B, N, D), np.float32)
    for c in range(NCORES):
        acc[c // 4] += np.asarray(res.results[c]["out"], dtype=np.float32)
    acc += np.asarray(bo)[None, None, :]
    return acc


# revision 69
# speedup vs baseline: 1.0421x; 1.0077x over previous
"""Causal self-attention (k/q swapped variant) on 8 Trainium2 NeuronCores.

Problem (hardcoded shapes): B=2, N=2048, D=1024, H=16, DH=64.
  kqv = einsum('bnd,hde->bhne', x, Wkqv) + bkqv   ; split -> k, q, v
  A[b,h,n,m] = k[b,h,n]·q[b,h,m] / sqrt(DH), causal mask m<=n, softmax over m
  sa = A @ v ; concat heads ; out = sa @ Wo + bo

Sharding: batch x heads — core c owns batch c//4 and heads 4*(c%4)..+4 (two
head-pairs A/B), computes its partial output projection sa_local @ Wo[rows]
in bf16 over its single batch, and the host sums 4 partials per batch (+bo)
in fp32. This halves both the x input DMA and the partial-output DMA vs
all-batches-per-core head sharding.

Per-core device kernel (all matmul operands bf16, fp32 PSUM accumulation):
  - x is pre-transposed on host to xt = x[b].T ([D, N]) so the contraction
    dim d lands on SBUF partitions; one copy shared by both head-pairs.
  - k/q are projected with W-stationary matmuls into [dh, n] layout; v is
    projected directly into [n, dh] layout with xt-stationary matmuls (one
    matmul covers all 4 heads), so no PE transposes / extra copies at all.
  - scores are computed transposed, S^T[m, n] = q[m]·k[n], so softmax's
    reduction dim m sits on partitions; both heads of a pair live in ONE
    [128, 1024] PSUM tile (2 banks) so off-diagonal chunks need a single wide
    exp() on the ACT engine (diag chunks use one strided-AP exp). The
    denominator comes free from the PV matmul: v blocks are [v_h|ones] so
    PV psum rows 64:128 hold the softmax denominators for both heads.
  - the chunk loop is software-pipelined: scores+exp of chunk ci+1 are
    emitted before the PV of chunk ci; k/q projection and output-projection
    slots are woven between chunks; v-block projections are emitted inline
    in pair-0 blocks right where the PE would otherwise wait for exp.
  - pair-1 attention blocks run in DESCENDING j order so the final block is
    the 4-chunk j=0 and the tail (last norm + out-blocks 0..3) is short.
  - PSUM budget (8 banks): scores 2x[128,1024] (4) + PV accumulator
    [128,1024] (2) + proj/outproj bank 1 + v-proj/outproj bank 1.
"""

import numpy as np
import ml_dtypes

B = 2
N = 2048
D = 1024
H = 16
DH = 64
NCORES = 8
HL = 2                    # heads per pair
NPAIR = 2                 # head-pairs per core
DC = D // 128             # contraction chunks = 8
NB = N // 128             # 128-row blocks = 16
NJ = N // 512             # 512-col blocks = 4

BF16 = ml_dtypes.bfloat16

_CACHE = {}


def _build():
    import concourse.bass as bass
    import concourse.mybir as mybir
    import concourse.tile as tile
    from concourse import bacc
    from contextlib import ExitStack

    f32 = mybir.dt.float32
    bf16 = mybir.dt.bfloat16
    Exp = mybir.ActivationFunctionType.Exp

    nc = bacc.Bacc("TRN2", target_bir_lowering=False, debug=False,
                   enable_asserts=False, num_devices=NCORES)

    xt_d = nc.dram_tensor("xt", [D, N], bf16, kind="ExternalInput")
    # k/q weights arrive pre-shuffled to the SBUF layout [128, DC*128]
    # (partition = within-chunk row, free = (chunk, head-col)), per pair
    wk_d = {p: nc.dram_tensor(f"wk{p}", [128, DC * 128], bf16,
                              kind="ExternalInput") for p in range(NPAIR)}
    wq_d = {p: nc.dram_tensor(f"wq{p}", [128, DC * 128], bf16,
                              kind="ExternalInput") for p in range(NPAIR)}
    # v weights combined for both pairs: per chunk dc the 256 cols are
    # [pair0 h0|h1 | pair1 h0|h1]
    wvall_d = nc.dram_tensor("wvall", [128, DC * 256], bf16,
                             kind="ExternalInput")
    wo_d = {p: nc.dram_tensor(f"wo{p}", [128, D], bf16,
                              kind="ExternalInput") for p in range(NPAIR)}
    # packed f32 consts: [bvr0(128) | bvr1(128) | bk0 | bq0 | bk1 | bq1]
    cf32_d = nc.dram_tensor("cf32", [128, 260], f32, kind="ExternalInput")
    m01_d = nc.dram_tensor("m01", [128, 128], bf16, kind="ExternalInput")
    out_d = nc.dram_tensor("out", [N, D], bf16, kind="ExternalOutput")

    with tile.TileContext(nc) as tc, ExitStack() as ctx:
        const = ctx.enter_context(tc.tile_pool(name="const", bufs=1))
        xt_pool = ctx.enter_context(tc.tile_pool(name="xt", bufs=1))
        kq_pool = ctx.enter_context(tc.tile_pool(name="kq", bufs=4))
        v_pool = ctx.enter_context(tc.tile_pool(name="v", bufs=2))
        sa_pool = ctx.enter_context(tc.tile_pool(name="sa", bufs=2))
        pt_pool = ctx.enter_context(tc.tile_pool(name="pt", bufs=6))
        rc_pool = ctx.enter_context(tc.tile_pool(name="rc", bufs=2))
        ob_pool = ctx.enter_context(tc.tile_pool(name="ob", bufs=4))
        s_ps = ctx.enter_context(tc.tile_pool(name="s_ps", bufs=2, space="PSUM"))
        pv_ps = ctx.enter_context(tc.tile_pool(name="pv_ps", bufs=1, space="PSUM"))
        wv_ps = ctx.enter_context(tc.tile_pool(name="wv_ps", bufs=1, space="PSUM"))
        tp_ps = ctx.enter_context(tc.tile_pool(name="tp_ps", bufs=1, space="PSUM"))

        wk_sb, wq_sb, wo_sb = {}, {}, {}
        for p in range(NPAIR):
            wk_sb[p] = const.tile([128, DC * 128], bf16, name=f"wk_sb{p}")
            wq_sb[p] = const.tile([128, DC * 128], bf16, name=f"wq_sb{p}")
            wo_sb[p] = const.tile([128, D], bf16, name=f"wo_sb{p}")
        wvall_sb = const.tile([128, DC * 256], bf16, name="wvall_sb")
        cf32_sb = const.tile([128, 260], f32, name="cf32_sb")
        m01_sb = const.tile([128, 128], bf16, name="m01_sb")
        bvr = {p: cf32_sb[:, 128 * p:128 * p + 128] for p in range(NPAIR)}
        bk = {p: cf32_sb[:, 256 + 2 * p:257 + 2 * p] for p in range(NPAIR)}
        bq = {p: cf32_sb[:, 257 + 2 * p:258 + 2 * p] for p in range(NPAIR)}

        # ---- DMA issue order: what the start of the pipeline needs first.
        # kqv0 needs wk0 + xt q0; the first dvv needs wvall + bvr; the first
        # diag masks need m01; q1 feeds the first weave slots. Each xt
        # quadrant is ONE multi-dim DMA ([p, dc, c] pattern) — per-DMA ring
        # time (~0.6us) dominates piece transfers, so fewer, bigger is
        # faster. q0 is split in two (dc 0-3 / 4-7 over both rings) so the
        # kqv0 chain can start sooner.
        # q0 is TWO tiles (dc 0-3 / 4-7) so the kqv0 chain starts on the
        # first half while the second transfers (tile deps are whole-tile)
        xtq = {}   # quad -> [128, DC*512] tile (chunk dc at cols dc*512)
        xt0h = {}  # half -> [128, 4*512] tile (dc%4 at cols (dc%4)*512)

        def xt_quad(q, eng, dc0=0, dc1=DC, tile_=None):
            t = tile_
            if t is None:
                t = xt_pool.tile([128, DC * 512], bf16, name=f"xt_q{q}",
                                 tag="xt", bufs=5)
                xtq[q] = t
            eng.dma_start(
                t[:].rearrange("p (dc c) -> p dc c", c=512)[:, dc0:dc1, :],
                xt_d.ap()[:, q * 512:(q + 1) * 512]
                .rearrange("(dc p) c -> p dc c", p=128)[:, dc0:dc1, :])
            return t

        nc.scalar.dma_start(wk_sb[0][:], wk_d[0].ap())
        for half in range(2):
            t = xt_pool.tile([128, 4 * 512], bf16, name=f"xt_q0h{half}",
                             tag="xt", bufs=5)
            (nc.sync if half == 0 else nc.scalar).dma_start(
                t[:].rearrange("p (dc c) -> p dc c", c=512),
                xt_d.ap()[:, 0:512]
                .rearrange("(dc p) c -> p dc c", p=128)[:, 4 * half:4 * half + 4, :])
            xt0h[half] = t
        nc.sync.dma_start(wq_sb[0][:], wq_d[0].ap())
        nc.scalar.dma_start(cf32_sb[:], cf32_d.ap())
        # wvall split over both rings so dvv(0) (att_j(0,0) chunk 0, ~t+8us)
        # isn't stuck behind a 1MB transfer on one ring
        nc.sync.dma_start(wvall_sb[:, 0:DC * 128],
                          wvall_d.ap()[:, 0:DC * 128])
        nc.scalar.dma_start(wvall_sb[:, DC * 128:DC * 256],
                            wvall_d.ap()[:, DC * 128:DC * 256])
        nc.sync.dma_start(m01_sb[:], m01_d.ap())
        xt_quad(1, nc.sync)
        xt_quad(2, nc.sync)
        nc.sync.dma_start(wk_sb[1][:], wk_d[1].ap())
        nc.sync.dma_start(wq_sb[1][:], wq_d[1].ap())
        xt_quad(3, nc.sync)
        nc.sync.dma_start(wo_sb[0][:], wo_d[0].ap())
        nc.sync.dma_start(wo_sb[1][:], wo_d[1].ap())

        def xt_ap(dc, c0, c1):
            q = c0 // 512
            if q == 0:
                return xt0h[dc // 4][:, (dc % 4) * 512 + c0:
                                     (dc % 4) * 512 + c1]
            return xtq[q][:, dc * 512 + c0 - q * 512:dc * 512 + c1 - q * 512]

        # ---- per-pair tensors
        k2 = {p: kq_pool.tile([128, N], bf16, name=f"k2_p{p}", tag="kq")
              for p in range(NPAIR)}
        q2 = {p: kq_pool.tile([128, N], bf16, name=f"q2_p{p}", tag="kq")
              for p in range(NPAIR)}
        # v blocks per nb: [ones(64) | v_h0(64) | ones(64) | v_h1(64)] so the
        # PV lhsT for head h is the contiguous 128 cols at nb*256 + 128h and
        # psum rows 0:64 give the denominator for BOTH heads (base-0 rows:
        # reciprocal_approx_fast is a custom-DVE op and misreads partition-
        # offset operands on HW).
        v_sb = {}
        sa_sb = {}
        groups = {p: ((wk_sb[p], bk[p], k2[p]), (wq_sb[p], bq[p], q2[p]))
                  for p in range(NPAIR)}

        for p in range(NPAIR):
            v_sb[p] = v_pool.tile([128, NB * 256], bf16, name=f"v_p{p}",
                                  tag="v")
            sa_sb[p] = sa_pool.tile([128, N], bf16, name=f"sa_p{p}", tag="sa")
            nc.gpsimd.memset(
                v_sb[p][:].rearrange("p (nb h q) -> p nb h q", h=2, q=128)
                [:, :, :, 0:64], 1.0)

        def make_proj_slots(p, gi, nj):
            """One [128, 512] k/q projection group as two weave slots (dc 0-3
            and dc 4-7 + bias add), sharing the wv PSUM bank."""
            st = {}

            def part(d0, d1):
                def go():
                    if d0 == 0:
                        st["ps"] = wv_ps.tile([128, 512], f32, name="wvps",
                                              tag="wv")
                    ps = st["ps"]
                    w_sb, bias_ap, dst = groups[p][gi]
                    for dc in range(d0, d1):
                        nc.tensor.matmul(
                            ps[:], w_sb[:, dc * 128:(dc + 1) * 128],
                            xt_ap(dc, nj * 512, (nj + 1) * 512),
                            start=(dc == 0), stop=(dc == DC - 1))
                    if d1 == DC:
                        nc.vector.tensor_scalar_add(
                            dst[:, nj * 512:(nj + 1) * 512], ps[:], bias_ap)
                return go
            return [part(0, 4), part(4, DC)]

        def make_dvv(nb):
            """v projection for one 128-row block, all 4 heads in one
            xt-stationary matmul chain; strided DVE write adds the bias and
            drops the result into the [v|ones|v|ones] layout."""
            vps = tp_ps.tile([128, 256], f32, name="vps", tag="tp",
                             padded_shape=[128, 512])
            q, r = nb // 4, nb % 4
            for dc in range(DC):
                nc.tensor.matmul(vps[:],
                                 xt_ap(dc, q * 512 + r * 128,
                                       q * 512 + (r + 1) * 128),
                                 wvall_sb[:, dc * 256:(dc + 1) * 256],
                                 start=(dc == 0), stop=(dc == DC - 1))
            for p in range(NPAIR):
                nc.vector.tensor_tensor(
                    v_sb[p][:, nb * 256:(nb + 1) * 256]
                    .rearrange("p (h q2) -> p h q2", q2=128)[:, :, 64:128],
                    vps[:, 128 * p:128 * p + 128]
                    .rearrange("p (h e) -> p h e", e=64),
                    bvr[p].rearrange("p (h e) -> p h e", e=64),
                    mybir.AluOpType.add)

        def make_op_slot(nb, dma_eng=None, cast_eng=None, alt_pool=False):
            """Output projection of one 128-row block: both pairs accumulate;
            the two column halves alternate over the wv / tp banks so casts
            overlap the next matmuls; one [128, 1024] bf16 DMA per block.
            Casts default to DVE; tail blocks use ACT (idle by then). Tail
            blocks (alt_pool) rotate odd blocks through the freed score
            banks so the serialized casts never gate the next matmuls."""
            def go():
                ob = ob_pool.tile([128, 1024], bf16, name="ob", tag="ob")
                for half in range(2):
                    if alt_pool and nb % 2 == 1:
                        pool, tag, pshape = s_ps, "s", [128, 1024]
                    else:
                        pool, tag, pshape = ((wv_ps, "wv", [128, 512])
                                             if half == 0 else
                                             (tp_ps, "tp", [128, 512]))
                    op = pool.tile([128, 512], f32, name="opps", tag=tag,
                                   padded_shape=pshape)
                    for p in range(NPAIR):
                        nc.tensor.matmul(
                            op[:], sa_sb[p][:, nb * 128:(nb + 1) * 128],
                            wo_sb[p][:, half * 512:(half + 1) * 512],
                            start=(p == 0), stop=(p == NPAIR - 1))
                    ce = cast_eng
                    if ce == "mix":
                        ce = nc.vector if half == 0 else nc.scalar
                    if ce is nc.scalar:
                        nc.scalar.copy(ob[:, half * 512:(half + 1) * 512],
                                       op[:])
                    else:
                        nc.vector.tensor_copy(
                            ob[:, half * 512:(half + 1) * 512], op[:])
                (dma_eng or nc.sync).dma_start(
                    out_d.ap()[nb * 128:(nb + 1) * 128, :], ob[:])
            return go

        def kqv0_start():
            """Pair-A k/q projections for nj=0, d-chunk-major so the PE is
            paced by the q0 xt piece DMAs; all k matmuls first so the PE can
            start before wq0 lands (k/q in one score-pool tile)."""
            kq0s = s_ps.tile([128, 1024], f32, name="kq0s", tag="s")
            for dc in range(DC):
                nc.tensor.matmul(kq0s[:, 0:512],
                                 wk_sb[0][:, dc * 128:(dc + 1) * 128],
                                 xt_ap(dc, 0, 512),
                                 start=(dc == 0), stop=(dc == DC - 1))
            nc.vector.tensor_scalar_add(k2[0][:, 0:512], kq0s[:, 0:512],
                                        bk[0])
            for dc in range(DC):
                nc.tensor.matmul(kq0s[:, 512:1024],
                                 wq_sb[0][:, dc * 128:(dc + 1) * 128],
                                 xt_ap(dc, 0, 512),
                                 start=(dc == 0), stop=(dc == DC - 1))
            nc.vector.tensor_scalar_add(q2[0][:, 0:512], kq0s[:, 512:1024],
                                        bq[0])

        def emit_scores(p, j, ci, state, mask_eng=None):
            """Scores (both heads into one 2-bank psum tile) + exp for one
            128-m chunk."""
            t = ci - 4 * j
            lo = 128 * t if t >= 0 else 0
            sp = s_ps.tile([128, 1024], f32, name="s", tag="s")
            pt = pt_pool.tile([128, 1024], bf16, name="pt", tag="pt")
            for h in range(HL):
                hp = 64 * h
                nc.tensor.matmul(
                    sp[:, 512 * h + lo:512 * h + 512],
                    q2[p][hp:hp + 64, ci * 128:(ci + 1) * 128],
                    k2[p][hp:hp + 64, j * 512 + lo:(j + 1) * 512],
                    start=True, stop=True)
            if t < 0:
                nc.scalar.activation(pt[:], sp[:], Exp, scale=0.125)
            else:
                # one strided activation covering both heads' valid region
                nc.scalar.activation(
                    pt[:].rearrange("p (h w) -> p h w", h=HL)[:, :, lo:512],
                    sp[:].rearrange("p (h w) -> p h w", h=HL)[:, :, lo:512],
                    Exp, scale=0.125)
                for h in range(HL):
                    # bf16 in/out: cheap on DVE; the last block's masks go to
                    # Pool instead so they don't queue behind the previous
                    # block's norm chain on DVE (bf16-only Pool ops are fine)
                    (mask_eng or nc.vector).tensor_tensor(
                        pt[:, 512 * h + lo:512 * h + lo + 128],
                        pt[:, 512 * h + lo:512 * h + lo + 128],
                        m01_sb[:], mybir.AluOpType.mult)
            state[ci] = (pt, lo)

        def emit_pv(p, j, ci, pv2, state, nch):
            pt, lo = state.pop(ci)
            for h in range(HL):
                nc.tensor.matmul(
                    pv2[:, 512 * h + lo:512 * (h + 1)],
                    v_sb[p][:, ci * 256 + 128 * h:ci * 256 + 128 * h + 128],
                    pt[:, 512 * h + lo:512 * h + 512],
                    start=(ci == 0), stop=(ci == nch - 1))

        def att_norm(p, j, pv2, last=False):
            # psum rows 0:64 = denominator, 64:128 = sa, for BOTH heads.
            # ONE full-tile copy (DVE cost is free-size only, so copying all
            # 128 rows costs the same as 64) frees the PV psum in ~1.25us so
            # the next block's PV isn't WAR-blocked; the mults become
            # SBUF-only and run on the Pool engine, keeping the DVE queue
            # short ahead of the next block's diag masks.
            # denominators are sums of exp() in [~2e-3, ~3e3]: inside
            # approx_fast's domain; 18-bit accuracy is far below the bf16
            # noise of the P*V numerator. (approx_fast misreads PSUM
            # operands on HW - bounce through SBUF first.)
            den_sb = rc_pool.tile([64, 1024], f32, name="den", tag="den")
            nc.vector.tensor_copy(den_sb[:], pv2[0:64, :])
            if last:
                # no successor uses the pv psum bank: read sa straight from
                # PSUM (exempt from the same-start-partition rule), saving
                # one copy on the tail critical path
                saf = pv2[64:128, :]
            else:
                saf_t = rc_pool.tile([64, 1024], f32, name="saf", tag="saf")
                nc.vector.tensor_copy(saf_t[:], pv2[64:128, :])
                saf = saf_t[:]
            rc = rc_pool.tile([64, 1024], f32, name="rc", tag="rc")
            nc.vector.reciprocal_approx_fast(rc[:], den_sb[:])
            for h in range(HL):
                # fp32-in/bf16-out tensor_tensor gives wrong results on the
                # Pool engine on HW (sim is fine) - keep these on DVE; all
                # SBUF inputs are base-partition-0 (verifier requirement,
                # and custom-DVE ops misread partition-offset operands)
                nc.vector.tensor_tensor(
                    sa_sb[p][64 * h:64 * h + 64, j * 512:(j + 1) * 512],
                    saf[:, 512 * h:512 * (h + 1)],
                    rc[:, 512 * h:512 * (h + 1)], mybir.AluOpType.mult)

        def att_j(p, j, weave=(), dv=False, wd=0, late=None, defer_norm=False,
                  mask_eng=None, last=False):
            """One attention j-block, software-pipelined: scores/exp of chunk
            ci+1 are emitted before PV of chunk ci; weave slots fill PE gaps
            (starting at chunk `wd` — op-slot weaves depend on the previous
            block's att_norm, so give that DVE chain a head start); in pair-0
            blocks the v projections for this j's rows are emitted inline
            where the PE would wait for exp. `late` (e.g. the previous
            block's deferred norm) is emitted after chunk 1 so this block's
            first diag masks aren't queued behind it on DVE. With
            `defer_norm` the caller gets a closure to emit this block's norm
            later."""
            pv2 = pv_ps.tile([128, 1024], f32, name="pv2", tag="pv")
            weave = list(weave)
            nch = 4 * (j + 1)
            state = {}
            emitted = 0
            emit_scores(p, j, 0, state, mask_eng=mask_eng)
            for ci in range(nch):
                if ci + 1 < nch:
                    emit_scores(p, j, ci + 1, state, mask_eng=mask_eng)
                if ci == 1 and late is not None:
                    late()
                if ci >= wd:
                    target = len(weave) * (ci + 1 - wd) // (nch - wd)
                    while emitted < target:
                        weave[emitted]()
                        emitted += 1
                if dv and ci < 4:
                    make_dvv(4 * j + ci)
                emit_pv(p, j, ci, pv2, state, nch)
            # norm first so its DVE chain heads the queue; leftover weave
            # slots (PE/ACT work) then overlap it
            if defer_norm:
                norm = [lambda: att_norm(p, j, pv2, last=last)]
            else:
                att_norm(p, j, pv2, last=last)
                norm = []
            while emitted < len(weave):
                weave[emitted]()
                emitted += 1
            return norm[0] if norm else None

        # ================= emission schedule =================
        def ksl(p, nj):
            return make_proj_slots(p, 0, nj)

        def qsl(p, nj):
            return make_proj_slots(p, 1, nj)

        def ops(nbs, cast_eng=None, alt_pool=False):
            return [make_op_slot(nb, cast_eng=cast_eng, alt_pool=alt_pool)
                    for nb in nbs]

        kqv0_start()
        att_j(0, 0, weave=ksl(0, 1) + qsl(0, 1), dv=True)
        att_j(0, 1, weave=ksl(0, 2) + qsl(0, 2), dv=True)
        # (1,3)'s scores read ALL of q2[1], so every qsl(1,*) must precede
        # it; only the k-blocks of later pair-1 js may ride inside it
        att_j(0, 2, weave=ksl(0, 3) + qsl(0, 3) + qsl(1, 0) + qsl(1, 1),
              dv=True)
        att_j(0, 3, weave=qsl(1, 2) + qsl(1, 3) + ksl(1, 3) + ksl(1, 2),
              dv=True)
        # pair-1 j-blocks DESCEND so the last block (and tail) is small;
        # ops() weaves start wd chunks in: the previous block's att_norm
        # (whose sa they read) is a ~5.6us DVE chain behind the last diag
        # masks, so firing an op matmul earlier stalls the in-order PE queue
        att_j(1, 3, weave=ksl(1, 1) + ksl(1, 0))
        att_j(1, 2, weave=ops(range(12, 16)), wd=5)
        att_j(1, 1, weave=ops(range(8, 12)), wd=6)
        # wd=4 > nch pushes the whole weave to the post-norm flush: op(4..7)
        # matmuls (ACT casts) overlap norm(1,0) on DVE
        att_j(1, 0, weave=ops(range(4, 8), cast_eng=nc.scalar),
              wd=4, mask_eng=nc.gpsimd, last=True)
        # tail: alternate the last out-DMAs over the two HW-DGE queues and
        # split casts across DVE/ACT so neither serializes the drain
        for nb, eng in zip(range(0, 4),
                           (nc.sync, nc.scalar, nc.sync, nc.scalar)):
            make_op_slot(nb, dma_eng=eng, cast_eng="mix")()

    nc.compile()
    return nc


def _get_nc():
    if "nc" not in _CACHE:
        _CACHE["nc"] = _build()
    return _CACHE["nc"]


def _prep_inputs(x, Wkqv, bkqv, Wo, bo):
    """Host-side shard prep: one input map per core (core c: batch c//4,
    head-pairs (4*(c%4), 4*(c%4)+1) and (+2, +3))."""
    tri = np.triu(np.ones((128, 128), np.float32)).astype(BF16)  # m' <= n''
    xts = [np.ascontiguousarray(x[b].T).astype(BF16) for b in range(B)]

    def shuf(w):
        # [D, 128] -> [128, DC*128]: partition = within-chunk row
        return np.ascontiguousarray(
            w.reshape(DC, 128, 128).transpose(1, 0, 2).reshape(128, DC * 128))

    in_maps = []
    for c in range(NCORES):
        b, m = c // 4, c % 4
        im = {"xt": xts[b], "m01": tri}
        wvs = {}
        cf32 = np.zeros((128, 260), np.float32)
        for p in range(NPAIR):
            h0 = 4 * m + 2 * p
            h1 = h0 + 1
            im[f"wk{p}"] = shuf(np.concatenate(
                [Wkqv[h0, :, 0:64], Wkqv[h1, :, 0:64]], axis=1)).astype(BF16)
            im[f"wq{p}"] = shuf(np.concatenate(
                [Wkqv[h0, :, 64:128], Wkqv[h1, :, 64:128]], axis=1)).astype(BF16)
            wvs[p] = shuf(np.concatenate(
                [Wkqv[h0, :, 128:192], Wkqv[h1, :, 128:192]], axis=1)).astype(BF16)
            im[f"wo{p}"] = Wo[64 * h0:64 * h0 + 128, :].astype(BF16)
            bv_pair = np.concatenate([bkqv[h0, 128:192], bkqv[h1, 128:192]])
            cf32[:, 128 * p:128 * p + 128] = bv_pair[None, :]
            cf32[:, 256 + 2 * p] = np.concatenate(
                [bkqv[h0, 0:64], bkqv[h1, 0:64]])
            cf32[:, 257 + 2 * p] = np.concatenate(
                [bkqv[h0, 64:128], bkqv[h1, 64:128]])
        # combined v weights: per chunk dc, 256 cols = [pair0 128 | pair1 128]
        wvall = np.zeros((128, DC * 256), np.float32).astype(BF16)
        for dc in range(DC):
            wvall[:, dc * 256:dc * 256 + 128] = wvs[0][:, dc * 128:(dc + 1) * 128]
            wvall[:, dc * 256 + 128:dc * 256 + 256] = wvs[1][:, dc * 128:(dc + 1) * 128]
        im["wvall"] = wvall
        im["cf32"] = cf32
        in_maps.append(im)
    return in_maps


def kernel(x, Wkqv, bkqv, Wo, bo):
    from concourse import bass_utils

    nc = _get_nc()
    in_maps = _prep_inputs(np.asarray(x), np.asarray(Wkqv), np.asarray(bkqv),
                           np.asarray(Wo), np.asarray(bo))
    res = bass_utils.run_bass_kernel_spmd(nc, in_maps, core_ids=list(range(NCORES)))
    acc = np.zeros((B, N, D), np.float32)
    for c in range(NCORES):
        acc[c // 4] += np.asarray(res.results[c]["out"], dtype=np.float32)
    acc += np.asarray(bo)[None, None, :]
    return acc
